# revision 15
# baseline (speedup 1.0000x reference)
"""Trainium2 Bass kernel for nn_BottleneckTransformer.

Data-parallel over batch: B=16 samples -> 8 cores x 2 samples.
Key optimizations over the f32r/bf16 baseline:
  - scores^T and PV matmuls in fp8e4 with DoubleRow perf mode; q/k stored
    plane-packed, v^T stored with 80-wide head slots.
  - softmax exp split across ACT (table exp) and DVE (custom EXP32Q_ANT,
    p=(m^2+c)^32 ~ e^(s-1.2)); the e^-1.2 shift avoids fp8e4 overflow and
    cancels in the sumexp normalization.
  - k-bias dropped (constant per query -> cancels in softmax); v-bias folded
    into the conv2 shift on the host (softmax rows sum to 1).
  - sumexp reciprocal reads PSUM directly (no ACT-side row copy).
  - x, conv1 weights, CBAM spatial chain in bf16; final fuse on DVE with
    bf16 2x modes; conv2 z-add + channel sums on ACT via accum_out.
"""
import numpy as np
import ml_dtypes

import concourse.bacc as bacc
import concourse.bass as bass
import concourse.tile as tile
from concourse import mybir, bass_isa
from concourse.bass_utils import run_bass_kernel_spmd

# ---- custom DVE op: p = ((x + c0)^2 + c1)^32 -------------------------------
from concourse.dve_spec import Spec, Src0, C0, C1, Bin, AluOp, lower as _dve_lower
from concourse.dve_uop import DveOpSpec
import concourse.dve_ops as _dve_ops_mod
from concourse.dve_ops import DveOp as _DveOp


def _ref_exp32q(in0, in1, c0, c1, c2):
    f32 = np.float32
    t = in0.astype(f32)
    c0a = np.asarray(c0, f32)
    c1a = np.asarray(c1, f32)
    m = (t + c0a).astype(f32)
    p = (m * m).astype(f32)
    p = (p + c1a).astype(f32)
    for _ in range(5):
        p = (p * p).astype(f32)
    return p


def _register_exp32q():
    if "EXP32Q_ANT" in _dve_ops_mod.CUSTOM_DVE_SPECS:
        for op in _dve_ops_mod.OPS:
            if op.name == "EXP32Q_ANT":
                return op
    m = Src0 + C0
    p = Bin(AluOp.MULTIPLY, m, m) + C1
    for _ in range(5):
        p = Bin(AluOp.MULTIPLY, p, p)
    spec = Spec(body=p, reference=_ref_exp32q)
    row = _dve_ops_mod._CUSTOM_DVE_ROW_BASE + len(_dve_ops_mod.OPS)
    shas = {}
    for ver in ("v3", "v4"):
        uops = _dve_lower(spec, ver=ver)
        shas[ver] = DveOpSpec(name="EXP32Q_ANT", opcode=row, uops=uops,
                              rd1_en=False).sha(ver)
    op = _DveOp("EXP32Q_ANT", spec, subdim=False, uops_sha=shas)
    _dve_ops_mod.OPS.append(op)
    _dve_ops_mod.CUSTOM_DVE_SPECS[op.name] = op.spec
    _dve_ops_mod._SUB_OPCODE_FOR_NAME[op.name] = row
    return op


EXP32Q_ANT = _register_exp32q()

F32 = mybir.dt.float32
F32R = mybir.dt.float32r
BF16 = mybir.dt.bfloat16
FP8 = mybir.dt.float8e4

B, C, H, W = 16, 256, 32, 32
N = H * W          # 1024
NCORES = 8
SPC = B // NCORES  # samples per core = 2
NH, D = 4, 64      # heads, head dim
HS = 80            # padded head slot in v^T (16B-aligned stationary stride)
R = C // 8         # 32, channel attention bottleneck
EPS = 1e-5
DR = mybir.MatmulPerfMode.DoubleRow

# exp scaling: scores arrive as x = A2*(q.k) with A2 = sqc*ga/256;
# DVE computes ((x+C0)^2+C1)^32 = e^(qk/8 - 1.2); ACT uses
# exp(SCALE_ACT*x - 1.2).  The e^-1.2 shift cancels in normalization.
SHIFT = 1.2
SQC = float(np.exp(-SHIFT / 64.0))        # c^(1/2) with c = e^(-SHIFT/32)
GA = float(np.sqrt(2.0) / 2.0)
EXP_C0 = float(np.float32(SQC * GA))
EXP_C1 = float(np.float32(0.5 * np.exp(-SHIFT / 32.0)))
A2 = SQC * GA / 256.0                     # per-pair scale on q.k
ALPHA = float(np.sqrt(A2))                # fold into each of wq,bq,wk
SCALE_ACT = float(np.float32(32.0 / (SQC * GA)))

import os
# key blocks (mb) whose exp runs on the DVE custom op; rest on ACT
DVE_MBS = set(int(c) for c in os.environ.get("KNOB_DVE_MBS", "37"))


def build_module():
    nc = bacc.Bacc("TRN2", target_bir_lowering=False, debug=False)

    def din(name, shape, dt=F32):
        return nc.dram_tensor(name, shape, dt, kind="ExternalInput").ap()

    def dout(name, shape, dt=F32):
        return nc.dram_tensor(name, shape, dt, kind="ExternalOutput").ap()

    xr = din("xr", (SPC, 2, 128, N), BF16)        # per-sample x, c-blocks
    w1fT = din("w1fT", (2, 128, C), BF16)         # [kb][c,128 -> o cols]
    wqT = din("wqT", (2, 128, C), BF16)           # ALPHA-scaled
    wkT = din("wkT", (2, 128, C), BF16)
    wvT = din("wvT", (2, 128, C), BF16)           # gamma folded
    w2fT = din("w2fT", (2, 128, C), BF16)
    shift1 = din("shift1", (2, 128, 1), F32)
    bq_c = din("bq_c", (2, 128, 1), F32)          # per-plane bias (scaled)
    shift2 = din("shift2", (2, 128, 1), F32)      # bn2 shift + w2f@bvg
    caw1T = din("caw1T", (2, 128, 2 * R), F32)    # cols 0:32 avg(/1024), 32:64 max
    caw2T = din("caw2T", (R, C), F32)
    wband = din("wband", (32, 14, 32), BF16)      # sa conv bands, (c2,kx)
    ones_in = din("ones_in", (128, 1), BF16)

    out = dout("out", (SPC, 2, 128, N), F32)
    scr_sa = dout("scr_sa", (SPC, 32, 32), BF16)  # bounce: sigmoid(sa)

    with tile.TileContext(nc) as tc:
        with (
            tc.tile_pool(name="wpool", bufs=1) as wp,
            tc.tile_pool(name="xpool", bufs=1) as xp,
            tc.tile_pool(name="rpool", bufs=1) as rp,
            tc.tile_pool(name="qkpool", bufs=1) as qkp,
            tc.tile_pool(name="vpool", bufs=1) as vp,
            tc.tile_pool(name="epool", bufs=13) as ep,
            tc.tile_pool(name="ypool", bufs=1) as yp,
            tc.tile_pool(name="zpool", bufs=2) as zp,
            tc.tile_pool(name="spool", bufs=2) as sp,
            tc.tile_pool(name="opool", bufs=2) as op_,
            tc.tile_pool(name="ps_sc", bufs=2, space="PSUM") as ps_sc,
            tc.tile_pool(name="ps_at", bufs=1, space="PSUM") as ps_at,
            tc.tile_pool(name="ps_a", bufs=2, space="PSUM") as ps_a,
        ):
            # ---- load order: conv1 weights + inputs first ----
            w1t = wp.tile([128, 2, C], BF16, tag="w1t", name="w1t")
            nc.sync.dma_start(out=w1t, in_=w1fT.rearrange("k p c -> p k c"))
            sh1 = wp.tile([128, 2], F32, tag="sh1", name="sh1")
            nc.sync.dma_start(out=sh1, in_=shift1.rearrange("k p a -> p (k a)"))
            xt_all = [[xp.tile([128, N], BF16, tag=f"x{si}{cb}", name=f"x{si}{cb}")
                       for cb in range(2)] for si in range(SPC)]
            for nch in range(2):
                for cb in range(2):
                    nc.sync.dma_start(
                        out=xt_all[0][cb][:, nch * 512:(nch + 1) * 512],
                        in_=xr[0, cb][:, nch * 512:(nch + 1) * 512])
            wqt = wp.tile([128, 2, C], BF16, tag="wqt", name="wqt")
            nc.sync.dma_start(out=wqt, in_=wqT.rearrange("k p c -> p k c"))
            wkt = wp.tile([128, 2, C], BF16, tag="wkt", name="wkt")
            nc.sync.dma_start(out=wkt, in_=wkT.rearrange("k p c -> p k c"))
            for cb in range(2):
                nc.sync.dma_start(out=xt_all[1][cb], in_=xr[1, cb])
            bqc = wp.tile([128, 2], F32, tag="bqc", name="bqc")
            nc.sync.dma_start(out=bqc, in_=bq_c.rearrange("k p a -> p (k a)"))
            wvt = wp.tile([128, 2, C], BF16, tag="wvt", name="wvt")
            nc.sync.dma_start(out=wvt, in_=wvT.rearrange("k p c -> p k c"))
            w2t = wp.tile([128, 2, C], BF16, tag="w2t", name="w2t")
            nc.sync.dma_start(out=w2t, in_=w2fT.rearrange("k p c -> p k c"))
            sh2 = wp.tile([128, 2], F32, tag="sh2", name="sh2")
            nc.sync.dma_start(out=sh2, in_=shift2.rearrange("k p a -> p (k a)"))
            cw1 = wp.tile([128, 2, 2 * R], F32, tag="cw1", name="cw1")
            nc.sync.dma_start(out=cw1, in_=caw1T.rearrange("k p c -> p k c"))
            cw2 = wp.tile([R, C], F32, tag="cw2", name="cw2")
            nc.sync.dma_start(out=cw2, in_=caw2T)
            wbd = wp.tile([32, 14, 32], BF16, tag="wbd", name="wbd")
            nc.sync.dma_start(out=wbd, in_=wband)
            ones_fr = wp.tile([128, 1], BF16, tag="ones_fr", name="ones_fr")
            nc.sync.dma_start(out=ones_fr, in_=ones_in)
            nbias = wp.tile([128, 1], F32, tag="nbias", name="nbias")
            nc.vector.memset(nbias, -SHIFT)

            xt = [None] * SPC      # [s][cb] bf16 input tiles
            rt = [None] * SPC      # relu(conv1) tiles (bf16)
            qt = [None] * SPC      # [128, N] fp8 per ob
            kt = [None] * SPC
            vt = [None] * SPC      # vT fp8 [128, mb, NH*HS]
            ytmp = [None] * SPC    # y = attn + r (bf16)

            def a_conv(s):
                xt[s] = xt_all[s]
                rt[s] = [rp.tile([128, N], BF16, tag=f"r{s}{ob}", name=f"r{s}{ob}")
                         for ob in range(2)]
                for ob in range(2):
                    for nch in range(2):
                        pa = ps_a.tile([128, 512], F32, tag="pa", name="pa")
                        for kb in range(2):
                            nc.tensor.matmul(
                                pa, w1t[:, kb, ob * 128:(ob + 1) * 128],
                                xt[s][kb][:, nch * 512:(nch + 1) * 512],
                                start=(kb == 0), stop=(kb == 1))
                        nc.scalar.activation(
                            out=rt[s][ob][:, nch * 512:(nch + 1) * 512],
                            in_=pa, bias=sh1[:, ob:ob + 1], scale=1.0,
                            func=mybir.ActivationFunctionType.Relu)

            def a_qk(s, dst, wt, bc, on_act):
                for ob in range(2):
                    for nch in range(2):
                        pa = ps_a.tile([128, 512], F32, tag="pa", name="pa")
                        for kb in range(2):
                            nc.tensor.matmul(
                                pa, wt[:, kb, ob * 128:(ob + 1) * 128],
                                rt[s][kb][:, nch * 512:(nch + 1) * 512],
                                start=(kb == 0), stop=(kb == 1))
                        if on_act:
                            nc.scalar.activation(
                                out=dst[ob][:, nch * 512:(nch + 1) * 512],
                                in_=pa, bias=bc[:, ob:ob + 1], scale=1.0,
                                func=mybir.ActivationFunctionType.Identity)
                        else:
                            nc.vector.tensor_scalar(
                                dst[ob][:, nch * 512:(nch + 1) * 512], pa,
                                0.0, None, mybir.AluOpType.add)

            def a_q(s):
                qt[s] = [qkp.tile([128, N], FP8, tag=f"q{s}{ob}",
                                  name=f"q{s}{ob}") for ob in range(2)]
                a_qk(s, qt[s], wqt, bqc, True)

            def a_k(s):
                kt[s] = [qkp.tile([128, N], FP8, tag=f"k{s}{ob}",
                                  name=f"k{s}{ob}") for ob in range(2)]
                a_qk(s, kt[s], wkt, None, False)

            def a_vt(s):
                vt[s] = vp.tile([128, 8, NH * HS], FP8, tag=f"v{s}", name=f"v{s}")
                vv = vt[s].rearrange("p m (h e) -> p m h e", h=NH)
                nc.gpsimd.memset(vv[:, :, :, D:D + 1], 1.0)
                nc.gpsimd.memset(vv[:, :, :, D + 1:HS], 0.0)
                for mb in range(8):
                    pa = ps_a.tile([128, 512], F32, tag="pa", name="pa")
                    for kb in range(2):
                        nc.tensor.matmul(
                            pa[:, 0:C],
                            rt[s][kb][:, mb * 128:(mb + 1) * 128],
                            wvt[:, kb, :], start=(kb == 0), stop=(kb == 1))
                    nc.vector.tensor_scalar(
                        vv[:, mb, :, 0:D],
                        pa[:, 0:C].rearrange("p (h d) -> p h d", h=NH),
                        0.0, None, mybir.AluOpType.add)

            def phase_a(s):
                a_conv(s)
                a_q(s)
                a_k(s)
                a_vt(s)

            def phase_b(s, extras=()):
                ytmp[s] = [yp.tile([128, N], BF16, tag=f"yt{s}{pb}",
                                   name=f"yt{s}{pb}")
                           for pb in range(2)]
                et_all = [[None] * 4 for _ in range(NH)]  # per (h, mb-pair)

                def emit_se(h, mb):
                    # scores^T for key block mb: out [128 keys, 1024 queries]
                    pb, ro = h // 2, (h % 2) * 64
                    dsl = slice(ro, ro + 64)
                    psc = ps_sc.tile([128, 1024], F32, tag="psc", name="psc")
                    for nch in range(2):
                        nc.tensor.matmul(
                            psc[:, nch * 512:(nch + 1) * 512],
                            kt[s][pb][dsl, mb * 128:(mb + 1) * 128],
                            qt[s][pb][dsl, nch * 512:(nch + 1) * 512],
                            start=True, stop=True)
                    pr, half = mb // 2, mb % 2
                    if half == 0:
                        et_all[h][pr] = ep.tile([128, 2048], FP8, tag="et",
                                                name="et")
                    dst = et_all[h][pr][:, half * 1024:(half + 1) * 1024]
                    if mb in DVE_MBS:
                        nc.vector._custom_dve(EXP32Q_ANT, out=dst, in0=psc,
                                              s0=EXP_C0, s1=EXP_C1)
                    else:
                        nc.scalar.activation(
                            out=dst, in_=psc,
                            func=mybir.ActivationFunctionType.Exp,
                            scale=SCALE_ACT, bias=nbias)

                def emit_pv(h):
                    pb, ro = h // 2, (h % 2) * 64
                    pat = ps_at.tile([HS, N], F32, tag="pat", name="pat")
                    for nch in range(2):
                        for pr in range(4):
                            nc.tensor.matmul(
                                pat[:, nch * 512:(nch + 1) * 512],
                                vt[s][:, 2 * pr:2 * pr + 2,
                                      h * HS:(h + 1) * HS],
                                et_all[h][pr].rearrange(
                                    "p (t n) -> p t n", t=2)[
                                    :, :, nch * 512:(nch + 1) * 512],
                                start=(pr == 0), stop=(pr == 3),
                                perf_mode=DR)
                    # normalize: recip of sumexp row (read straight from
                    # PSUM), broadcast, multiply
                    srow = sp.tile([1, N], F32, tag="srow", name="srow", bufs=1)
                    rr = sp.tile([1, N], F32, tag="rr", name="rr", bufs=1)
                    rb = sp.tile([64, N], F32, tag="rb", name="rb", bufs=1)
                    nc.scalar.copy(srow, pat[D:D + 1, :])
                    nc.vector.reciprocal_approx_fast(out=rr, in_=srow)
                    nc.gpsimd.partition_broadcast(rb, rr, channels=64)
                    nc.vector.tensor_tensor(
                        out=ytmp[s][pb][ro:ro + 64, :],
                        in0=pat[0:D, :], in1=rb, op=mybir.AluOpType.mult)

                PF = 6
                for h in range(NH):
                    for mb in (range(PF, 8) if h > 0 else range(8)):
                        emit_se(h, mb)
                    if h + 1 < NH:
                        for mb in range(PF):
                            emit_se(h + 1, mb)
                    emit_pv(h)
                    if h % 2 == 1:
                        # y = attn + r for finished pb (both bf16 -> DVE 2x)
                        pb = h // 2
                        nc.vector.tensor_tensor(
                            out=ytmp[s][pb], in0=ytmp[s][pb], in1=rt[s][pb],
                            op=mybir.AluOpType.add)
                    if extras and h < len(extras):
                        extras[h]()

            def c_conv(s):
                # conv2 + bn2 -> z (fp32) on ACT, with per-channel sums for
                # CBAM avg via accum_out
                zt = [zp.tile([128, N], F32, tag=f"z{ob}", name=f"z{ob}")
                      for ob in range(2)]
                cols = [zp.tile([128, 2], F32, tag=f"cols{ob}", name=f"cols{ob}")
                        for ob in range(2)]
                accs = [zp.tile([128, 2], F32, tag=f"acc{ob}", name=f"acc{ob}")
                        for ob in range(2)]
                for ob in range(2):
                    for nch in range(2):
                        pa = ps_a.tile([128, 512], F32, tag="pa", name="pa")
                        for kb in range(2):
                            nc.tensor.matmul(
                                pa, w2t[:, kb, ob * 128:(ob + 1) * 128],
                                ytmp[s][kb][:, nch * 512:(nch + 1) * 512],
                                start=(kb == 0), stop=(kb == 1))
                        if s == 1:
                            nc.scalar.activation(
                                out=zt[ob][:, nch * 512:(nch + 1) * 512],
                                in_=pa, bias=sh2[:, ob:ob + 1], scale=1.0,
                                func=mybir.ActivationFunctionType.Identity,
                                accum_out=accs[ob][:, nch:nch + 1])
                        else:
                            nc.vector.tensor_scalar(
                                zt[ob][:, nch * 512:(nch + 1) * 512], pa,
                                sh2[:, ob:ob + 1], 0.0,
                                mybir.AluOpType.add, mybir.AluOpType.add,
                                accum_out=accs[ob][:, nch:nch + 1])
                    # combine the two 512-chunk sums into col 0
                    nc.vector.tensor_tensor(
                        out=cols[ob][:, 0:1], in0=accs[ob][:, 0:1],
                        in1=accs[ob][:, 1:2], op=mybir.AluOpType.add)
                    # channel max into col 1
                    nc.vector.tensor_reduce(
                        out=cols[ob][:, 1:2], in_=zt[ob],
                        op=mybir.AluOpType.max,
                        axis=mybir.AxisListType.X)
                return zt, cols

            def c_chan(s, zt, cols):
                # channel attention: h = relu(W1a@sum | W1m@max), ca=sig(W2@h)
                ph_a = ps_a.tile([128, 512], F32, tag="pa", name="pa")
                for kb in range(2):
                    nc.tensor.matmul(ph_a[0:R, 0:1], cw1[:, kb, 0:R],
                                     cols[kb][:, 0:1],
                                     start=(kb == 0), stop=(kb == 1))
                ph_m = ps_a.tile([128, 512], F32, tag="pa", name="pa")
                for kb in range(2):
                    nc.tensor.matmul(ph_m[0:R, 0:1], cw1[:, kb, R:2 * R],
                                     cols[kb][:, 1:2],
                                     start=(kb == 0), stop=(kb == 1))
                hsb = sp.tile([R, 2], F32, tag="hsb", name="hsb")
                nc.vector.tensor_scalar(hsb[:, 0:1], ph_a[0:R, 0:1], 0.0, None,
                                        mybir.AluOpType.max)
                nc.vector.tensor_scalar(hsb[:, 1:2], ph_m[0:R, 0:1], 0.0, None,
                                        mybir.AluOpType.max)
                ca = [sp.tile([128, 1], F32, tag=f"ca{ob}", name=f"ca{ob}")
                      for ob in range(2)]
                for ob in range(2):
                    pc = ps_a.tile([128, 512], F32, tag="pa", name="pa")
                    nc.tensor.matmul(pc[:, 0:1], cw2[:, ob * 128:(ob + 1) * 128],
                                     hsb[:, 0:1], start=True, stop=False)
                    nc.tensor.matmul(pc[:, 0:1], cw2[:, ob * 128:(ob + 1) * 128],
                                     hsb[:, 1:2], start=False, stop=True)
                    nc.scalar.activation(
                        out=ca[ob], in_=pc[:, 0:1],
                        func=mybir.ActivationFunctionType.Tanh, scale=0.5)
                    nc.vector.tensor_scalar(
                        ca[ob], ca[ob], 0.5, 0.5,
                        mybir.AluOpType.mult, mybir.AluOpType.add)
                # apply channel attention -> z_ca (bf16)
                zca = [zp.tile([128, N], BF16, tag=f"zca{ob}", name=f"zca{ob}")
                       for ob in range(2)]
                for ob in range(2):
                    if s == 1:
                        nc.scalar.activation(
                            out=zca[ob], in_=zt[ob],
                            func=mybir.ActivationFunctionType.Copy,
                            scale=ca[ob])
                    else:
                        nc.vector.tensor_scalar_mul(zca[ob], zt[ob], ca[ob])
                return zca

            def c_spat(s, zca):
                # spatial sum (avg path) via ones matmul (bf16)
                avg_row = sp.tile([1, N], BF16, tag="avg_row", name="avg_row",
                                  bufs=1)
                for nch in range(2):
                    psr = ps_a.tile([128, 512], F32, tag="pa", name="pa")
                    for kb in range(2):
                        nc.tensor.matmul(
                            psr[0:1, :],
                            ones_fr, zca[kb][:, nch * 512:(nch + 1) * 512],
                            start=(kb == 0), stop=(kb == 1))
                    nc.scalar.copy(
                        avg_row[:, nch * 512:(nch + 1) * 512], psr[0:1, :])
                # spatial max via TT max (bf16 2x) + gpsimd reduce
                m1 = zp.tile([128, N], BF16, tag="m1", name="m1", bufs=1)
                nc.vector.tensor_tensor(out=m1, in0=zca[0], in1=zca[1],
                                        op=mybir.AluOpType.max)
                rep = zp.tile([128, N], BF16, tag="rep", name="rep", bufs=1)
                nc.gpsimd.partition_all_reduce(rep, m1, channels=128,
                                               reduce_op=bass_isa.ReduceOp.max)
                # reshape rows [1, 1024] -> [32(y), 32(x)] via SBUF->SBUF DMA
                avgT = sp.tile([32, 38], BF16, tag="avgT", name="avgT")
                nc.gpsimd.memset(avgT, 0.0)
                nc.sync.dma_start(out=avgT[:, 3:35], in_=avg_row)
                maxT = sp.tile([32, 38], BF16, tag="maxT", name="maxT")
                nc.gpsimd.memset(maxT, 0.0)
                nc.sync.dma_start(out=maxT[:, 3:35], in_=rep[0:1, :])
                # 7x7 conv as 14 banded matmuls over y, x-shifts on free dim
                psa = ps_a.tile([128, 512], F32, tag="pa", name="pa")
                first = True
                for c2, inp in ((0, avgT), (1, maxT)):
                    for kx in range(7):
                        nc.tensor.matmul(
                            psa[0:32, 0:32],
                            wbd[:, c2 * 7 + kx, :],
                            inp[:, kx:kx + 32],
                            start=first, stop=(c2 == 1 and kx == 6))
                        first = False
                sasb = sp.tile([32, 32], BF16, tag="sasb", name="sasb")
                nc.scalar.activation(
                    out=sasb, in_=psa[0:32, 0:32],
                    func=mybir.ActivationFunctionType.Tanh, scale=0.5)
                nc.vector.tensor_scalar(
                    sasb, sasb, 0.5, 0.5,
                    mybir.AluOpType.mult, mybir.AluOpType.add)
                sarow = sp.tile([1, N], BF16, tag="sarow", name="sarow")
                nc.sync.dma_start(out=sarow, in_=sasb)
                sarep = zp.tile([128, N], BF16, tag="sarep", name="sarep",
                                bufs=1)
                nc.gpsimd.partition_broadcast(sarep, sarow, channels=128)
                return sarep

            def c_fin(s, zca, sarep):
                # final: out = relu(z_ca * sa + x); cb0 TTs on DVE, cb1 TTs
                # on Pool so both channel blocks run in parallel in the tail
                us = [op_.tile([128, N], BF16, tag=f"fin{cb}", name=f"fin{cb}")
                      for cb in range(2)]
                os_ = [op_.tile([128, N], F32, tag=f"fino{cb}",
                                name=f"fino{cb}") for cb in range(2)]
                for ch in range(2):
                    lo, hi = ch * 512, ch * 512 + 512
                    for cb in range(2):
                        eng = nc.gpsimd if (cb == 1 and ch == 1) else nc.vector
                        u, o = us[cb], os_[cb]
                        eng.tensor_tensor(
                            out=u[:, lo:hi], in0=zca[cb][:, lo:hi],
                            in1=sarep[:, lo:hi], op=mybir.AluOpType.mult)
                        eng.tensor_tensor(
                            out=u[:, lo:hi], in0=u[:, lo:hi],
                            in1=xt[s][cb][:, lo:hi], op=mybir.AluOpType.add)
                        nc.vector.tensor_scalar(
                            o[:, lo:hi], u[:, lo:hi], 0.0, None,
                            mybir.AluOpType.max)
                        nc.sync.dma_start(out=out[s, cb][:, lo:hi],
                                          in_=o[:, lo:hi])

            def phase_c(s):
                zt, cols = c_conv(s)
                zca = c_chan(s, zt, cols)
                sarep = c_spat(s, zca)
                c_fin(s, zca, sarep)

            phase_a(0)
            phase_b(0, extras=(lambda: a_conv(1), lambda: a_q(1),
                               lambda: a_k(1), lambda: a_vt(1)))
            c0_state = {}

            def x0():
                c0_state["zt"], c0_state["cols"] = c_conv(0)

            def x1():
                c0_state["zca"] = c_chan(0, c0_state["zt"], c0_state["cols"])

            def x2():
                c0_state["sarep"] = c_spat(0, c0_state["zca"])

            def x3():
                c_fin(0, c0_state["zca"], c0_state["sarep"])

            phase_b(1, extras=(x0, x1, x2, x3))
            phase_c(1)

    nc.compile()
    return nc


_NC_CACHE = None


def get_module():
    global _NC_CACHE
    if _NC_CACHE is None:
        _NC_CACHE = build_module()
    return _NC_CACHE


def prep_inputs(x, w1, bn1_g, bn1_b, bn1_m, bn1_v, wq, bq, wk, bk, wv, bv,
                gamma, w2, bn2_g, bn2_b, bn2_m, bn2_v, ca_w1, ca_w2, sa_w):
    """Host-side preprocessing -> per-core in_maps."""
    f64 = np.float64
    bf16 = ml_dtypes.bfloat16
    s1 = (bn1_g.astype(f64) / np.sqrt(bn1_v.astype(f64) + EPS))
    w1f = (s1[:, None] * w1.astype(f64)).astype(np.float32)
    sh1 = (bn1_b.astype(f64) - bn1_m.astype(f64) * s1).astype(np.float32)
    s2 = (bn2_g.astype(f64) / np.sqrt(bn2_v.astype(f64) + EPS))
    w2f = (s2[:, None] * w2.astype(f64)).astype(np.float32)
    g = float(gamma[0])
    wvg = (wv.astype(f64) * g).astype(np.float32)
    bvg = (bv.astype(f64) * g)
    # v-bias folds into the conv2 shift: softmax rows sum to 1, so the
    # attention output gets exactly +bvg per channel before conv2.
    sh2 = (bn2_b.astype(f64) - bn2_m.astype(f64) * s2
           + (s2[:, None] * w2.astype(f64)) @ bvg).astype(np.float32)

    def lhsT(w):  # [O, C] -> [2, 128, O] kb-blocked transpose, bf16
        return np.ascontiguousarray(w.T.reshape(2, 128, C)).astype(bf16)

    def lhsT_qk(w):
        wt = (w.astype(f64) * ALPHA).astype(np.float32).T  # [C_in, O]
        return np.ascontiguousarray(wt.reshape(2, 128, C)).astype(bf16)

    def bias_qk(b):
        bb = (b.astype(f64) * ALPHA).astype(np.float32)
        return np.ascontiguousarray(bb.reshape(2, 128, 1))

    base = {
        "w1fT": lhsT(w1f),
        "wqT": lhsT_qk(wq),
        "wkT": lhsT_qk(wk),
        "wvT": np.ascontiguousarray(wvg.T.reshape(2, 128, C)).astype(bf16),
        "w2fT": lhsT(w2f),
        "shift1": np.ascontiguousarray(sh1.reshape(2, 128, 1)),
        "bq_c": bias_qk(bq),
        "shift2": np.ascontiguousarray(sh2.reshape(2, 128, 1)),
    }
    # channel attention weights: caw1T [2, 128, 64]
    c1T = ca_w1.T.astype(np.float32)             # [C, R]
    caw1T = np.concatenate([c1T / float(N), c1T], axis=1)  # [C, 2R]
    base["caw1T"] = np.ascontiguousarray(caw1T.reshape(2, 128, 2 * R))
    base["caw2T"] = np.ascontiguousarray(ca_w2.T.astype(np.float32))  # [R, C]
    # spatial conv bands: wband[yi, c2*7+kx, yo] = w[c2, yi-yo+3, kx]
    wb = np.zeros((32, 14, 32), np.float32)
    for c2 in range(2):
        for kx in range(7):
            for yo in range(32):
                for ky in range(7):
                    yi = yo + ky - 3
                    if 0 <= yi < 32:
                        v = sa_w[0, c2, ky, kx]
                        if c2 == 0:
                            v = v / float(C)
                        wb[yi, c2 * 7 + kx, yo] = v
    base["wband"] = wb.astype(bf16)
    base["ones_in"] = np.ones((128, 1), bf16)

    xrf = x.reshape(B, C, N).astype(bf16)
    in_maps = []
    for core in range(NCORES):
        m = dict(base)
        m["xr"] = np.ascontiguousarray(
            xrf[core * SPC:(core + 1) * SPC].reshape(SPC, 2, 128, N))
        in_maps.append(m)
    return in_maps


def kernel(**inputs):
    nc = get_module()
    in_maps = prep_inputs(**inputs)
    res = run_bass_kernel_spmd(nc, in_maps, core_ids=list(range(NCORES)))
    outs = []
    for core in range(NCORES):
        o = res.results[core]["out"]  # [SPC, 2, 128, N]
        outs.append(o.reshape(SPC, C, H, W))
    return np.concatenate(outs, axis=0)


if __name__ == "__main__":
    nc = get_module()
    print("compiled ok")


# revision 17
# speedup vs baseline: 1.0219x; 1.0219x over previous
"""Trainium2 Bass kernel for nn_BottleneckTransformer.

Data-parallel over batch: B=16 samples -> 8 cores x 2 samples.
Key optimizations over the f32r/bf16 baseline:
  - scores^T and PV matmuls in fp8e4 with DoubleRow perf mode; q/k stored
    plane-packed, v^T stored with 80-wide head slots.
  - softmax exp split across ACT (table exp) and DVE (custom EXP32Q_ANT,
    p=(m^2+c)^32 ~ e^(s-1.2)); the e^-1.2 shift avoids fp8e4 overflow and
    cancels in the sumexp normalization.
  - k-bias dropped (constant per query -> cancels in softmax); v-bias folded
    into the conv2 shift on the host (softmax rows sum to 1).
  - sumexp reciprocal reads PSUM directly (no ACT-side row copy).
  - x, conv1 weights, CBAM spatial chain in bf16; final fuse on DVE with
    bf16 2x modes; conv2 z-add + channel sums on ACT via accum_out.
"""
import numpy as np
import ml_dtypes

import concourse.bacc as bacc
import concourse.bass as bass
import concourse.tile as tile
from concourse import mybir, bass_isa
from concourse.bass_utils import run_bass_kernel_spmd

# ---- custom DVE op: p = ((x + c0)^2 + c1)^32 -------------------------------
from concourse.dve_spec import Spec, Src0, C0, C1, Bin, AluOp, lower as _dve_lower
from concourse.dve_uop import DveOpSpec
import concourse.dve_ops as _dve_ops_mod
from concourse.dve_ops import DveOp as _DveOp


def _ref_exp32q(in0, in1, c0, c1, c2):
    f32 = np.float32
    t = in0.astype(f32)
    c0a = np.asarray(c0, f32)
    c1a = np.asarray(c1, f32)
    m = (t + c0a).astype(f32)
    p = (m * m).astype(f32)
    p = (p + c1a).astype(f32)
    for _ in range(5):
        p = (p * p).astype(f32)
    return p


def _register_exp32q():
    if "EXP32Q_ANT" in _dve_ops_mod.CUSTOM_DVE_SPECS:
        for op in _dve_ops_mod.OPS:
            if op.name == "EXP32Q_ANT":
                return op
    m = Src0 + C0
    p = Bin(AluOp.MULTIPLY, m, m) + C1
    for _ in range(5):
        p = Bin(AluOp.MULTIPLY, p, p)
    spec = Spec(body=p, reference=_ref_exp32q)
    row = _dve_ops_mod._CUSTOM_DVE_ROW_BASE + len(_dve_ops_mod.OPS)
    shas = {}
    for ver in ("v3", "v4"):
        uops = _dve_lower(spec, ver=ver)
        shas[ver] = DveOpSpec(name="EXP32Q_ANT", opcode=row, uops=uops,
                              rd1_en=False).sha(ver)
    op = _DveOp("EXP32Q_ANT", spec, subdim=False, uops_sha=shas)
    _dve_ops_mod.OPS.append(op)
    _dve_ops_mod.CUSTOM_DVE_SPECS[op.name] = op.spec
    _dve_ops_mod._SUB_OPCODE_FOR_NAME[op.name] = row
    return op


EXP32Q_ANT = _register_exp32q()

F32 = mybir.dt.float32
F32R = mybir.dt.float32r
BF16 = mybir.dt.bfloat16
FP8 = mybir.dt.float8e4

B, C, H, W = 16, 256, 32, 32
N = H * W          # 1024
NCORES = 8
SPC = B // NCORES  # samples per core = 2
NH, D = 4, 64      # heads, head dim
HS = 80            # padded head slot in v^T (16B-aligned stationary stride)
R = C // 8         # 32, channel attention bottleneck
EPS = 1e-5
DR = mybir.MatmulPerfMode.DoubleRow

# exp scaling: scores arrive as x = A2*(q.k) with A2 = sqc*ga/256;
# DVE computes ((x+C0)^2+C1)^32 = e^(qk/8 - 1.2); ACT uses
# exp(SCALE_ACT*x - 1.2).  The e^-1.2 shift cancels in normalization.
SHIFT = 1.2
SQC = float(np.exp(-SHIFT / 64.0))        # c^(1/2) with c = e^(-SHIFT/32)
GA = float(np.sqrt(2.0) / 2.0)
EXP_C0 = float(np.float32(SQC * GA))
EXP_C1 = float(np.float32(0.5 * np.exp(-SHIFT / 32.0)))
A2 = SQC * GA / 256.0                     # per-pair scale on q.k
ALPHA = float(np.sqrt(A2))                # fold into each of wq,bq,wk
SCALE_ACT = float(np.float32(32.0 / (SQC * GA)))

import os
# key blocks (mb) whose exp runs on the DVE custom op; rest on ACT
DVE_MBS = set(int(c) for c in os.environ.get("KNOB_DVE_MBS", "357"))


def build_module():
    nc = bacc.Bacc("TRN2", target_bir_lowering=False, debug=False)

    def din(name, shape, dt=F32):
        return nc.dram_tensor(name, shape, dt, kind="ExternalInput").ap()

    def dout(name, shape, dt=F32):
        return nc.dram_tensor(name, shape, dt, kind="ExternalOutput").ap()

    xr = din("xr", (SPC, 2, 128, N), BF16)        # per-sample x, c-blocks
    w1fT = din("w1fT", (2, 128, C), BF16)         # [kb][c,128 -> o cols]
    wqT = din("wqT", (2, 128, C), BF16)           # ALPHA-scaled
    wkT = din("wkT", (2, 128, C), BF16)
    wvT = din("wvT", (2, 128, C), BF16)           # gamma folded
    w2fT = din("w2fT", (2, 128, C), BF16)
    shift1 = din("shift1", (2, 128, 1), F32)
    bq_c = din("bq_c", (2, 128, 1), F32)          # per-plane bias (scaled)
    shift2 = din("shift2", (2, 128, 1), F32)      # bn2 shift + w2f@bvg
    caw1T = din("caw1T", (2, 128, 2 * R), F32)    # cols 0:32 avg(/1024), 32:64 max
    caw2T = din("caw2T", (2 * R, C), F32)
    wband = din("wband", (32, 14, 32), BF16)      # sa conv bands, (c2,kx)
    ones_in = din("ones_in", (128, 1), BF16)

    out = dout("out", (SPC, 2, 128, N), F32)
    scr_sa = dout("scr_sa", (SPC, 32, 32), BF16)  # bounce: sigmoid(sa)

    with tile.TileContext(nc) as tc:
        with (
            tc.tile_pool(name="wpool", bufs=1) as wp,
            tc.tile_pool(name="xpool", bufs=1) as xp,
            tc.tile_pool(name="rpool", bufs=1) as rp,
            tc.tile_pool(name="qkpool", bufs=1) as qkp,
            tc.tile_pool(name="vpool", bufs=1) as vp,
            tc.tile_pool(name="epool", bufs=13) as ep,
            tc.tile_pool(name="ypool", bufs=1) as yp,
            tc.tile_pool(name="zpool", bufs=2) as zp,
            tc.tile_pool(name="spool", bufs=2) as sp,
            tc.tile_pool(name="opool", bufs=2) as op_,
            tc.tile_pool(name="ps_sc", bufs=2, space="PSUM") as ps_sc,
            tc.tile_pool(name="ps_at", bufs=1, space="PSUM") as ps_at,
            tc.tile_pool(name="ps_a", bufs=2, space="PSUM") as ps_a,
        ):
            # ---- load order: conv1 weights + inputs first ----
            w1t = wp.tile([128, 2, C], BF16, tag="w1t", name="w1t")
            nc.sync.dma_start(out=w1t, in_=w1fT.rearrange("k p c -> p k c"))
            sh1 = wp.tile([128, 2], F32, tag="sh1", name="sh1")
            nc.sync.dma_start(out=sh1, in_=shift1.rearrange("k p a -> p (k a)"))
            xt_all = [[xp.tile([128, N], BF16, tag=f"x{si}{cb}", name=f"x{si}{cb}")
                       for cb in range(2)] for si in range(SPC)]
            for nch in range(2):
                for cb in range(2):
                    nc.sync.dma_start(
                        out=xt_all[0][cb][:, nch * 512:(nch + 1) * 512],
                        in_=xr[0, cb][:, nch * 512:(nch + 1) * 512])
            wqt = wp.tile([128, 2, C], BF16, tag="wqt", name="wqt")
            nc.sync.dma_start(out=wqt, in_=wqT.rearrange("k p c -> p k c"))
            wkt = wp.tile([128, 2, C], BF16, tag="wkt", name="wkt")
            nc.sync.dma_start(out=wkt, in_=wkT.rearrange("k p c -> p k c"))
            for cb in range(2):
                nc.sync.dma_start(out=xt_all[1][cb], in_=xr[1, cb])
            bqc = wp.tile([128, 2], F32, tag="bqc", name="bqc")
            nc.sync.dma_start(out=bqc, in_=bq_c.rearrange("k p a -> p (k a)"))
            wvt = wp.tile([128, 2, C], BF16, tag="wvt", name="wvt")
            nc.sync.dma_start(out=wvt, in_=wvT.rearrange("k p c -> p k c"))
            w2t = wp.tile([128, 2, C], BF16, tag="w2t", name="w2t")
            nc.sync.dma_start(out=w2t, in_=w2fT.rearrange("k p c -> p k c"))
            sh2 = wp.tile([128, 2], F32, tag="sh2", name="sh2")
            nc.sync.dma_start(out=sh2, in_=shift2.rearrange("k p a -> p (k a)"))
            cw1 = wp.tile([128, 2, 2 * R], F32, tag="cw1", name="cw1")
            nc.sync.dma_start(out=cw1, in_=caw1T.rearrange("k p c -> p k c"))
            cw2 = wp.tile([2 * R, C], F32, tag="cw2", name="cw2")
            nc.sync.dma_start(out=cw2, in_=caw2T)
            wbd = wp.tile([32, 14, 32], BF16, tag="wbd", name="wbd")
            nc.sync.dma_start(out=wbd, in_=wband)
            ones_fr = wp.tile([128, 1], BF16, tag="ones_fr", name="ones_fr")
            nc.sync.dma_start(out=ones_fr, in_=ones_in)
            nbias = wp.tile([128, 1], F32, tag="nbias", name="nbias")
            nc.vector.memset(nbias, -SHIFT)

            xt = [None] * SPC      # [s][cb] bf16 input tiles
            rt = [None] * SPC      # relu(conv1) tiles (bf16)
            qt = [None] * SPC      # [128, N] fp8 per ob
            kt = [None] * SPC
            vt = [None] * SPC      # vT fp8 [128, mb, NH*HS]
            ytmp = [None] * SPC    # y = attn + r (bf16)

            def a_conv(s):
                xt[s] = xt_all[s]
                rt[s] = [rp.tile([128, N], BF16, tag=f"r{s}{ob}", name=f"r{s}{ob}")
                         for ob in range(2)]
                for ob in range(2):
                    for nch in range(2):
                        pa = ps_a.tile([128, 512], F32, tag="pa", name="pa")
                        for kb in range(2):
                            nc.tensor.matmul(
                                pa, w1t[:, kb, ob * 128:(ob + 1) * 128],
                                xt[s][kb][:, nch * 512:(nch + 1) * 512],
                                start=(kb == 0), stop=(kb == 1))
                        nc.scalar.activation(
                            out=rt[s][ob][:, nch * 512:(nch + 1) * 512],
                            in_=pa, bias=sh1[:, ob:ob + 1], scale=1.0,
                            func=mybir.ActivationFunctionType.Relu)

            def a_qk(s, dst, wt, bc, on_act):
                for ob in range(2):
                    for nch in range(2):
                        pa = ps_a.tile([128, 512], F32, tag="pa", name="pa")
                        for kb in range(2):
                            nc.tensor.matmul(
                                pa, wt[:, kb, ob * 128:(ob + 1) * 128],
                                rt[s][kb][:, nch * 512:(nch + 1) * 512],
                                start=(kb == 0), stop=(kb == 1))
                        if on_act:
                            nc.scalar.activation(
                                out=dst[ob][:, nch * 512:(nch + 1) * 512],
                                in_=pa,
                                bias=(bc[:, ob:ob + 1] if bc is not None
                                      else 0.0),
                                scale=1.0,
                                func=(mybir.ActivationFunctionType.Identity
                                      if bc is not None else
                                      mybir.ActivationFunctionType.Copy))
                        else:
                            nc.vector.tensor_scalar(
                                dst[ob][:, nch * 512:(nch + 1) * 512], pa,
                                0.0, None, mybir.AluOpType.add)

            def a_q(s):
                qt[s] = [qkp.tile([128, N], FP8, tag=f"q{s}{ob}",
                                  name=f"q{s}{ob}") for ob in range(2)]
                a_qk(s, qt[s], wqt, bqc, True)

            def a_k(s):
                kt[s] = [qkp.tile([128, N], FP8, tag=f"k{s}{ob}",
                                  name=f"k{s}{ob}") for ob in range(2)]
                a_qk(s, kt[s], wkt, None, True)

            def a_vt(s):
                vt[s] = vp.tile([128, 8, NH * HS], FP8, tag=f"v{s}", name=f"v{s}")
                vv = vt[s].rearrange("p m (h e) -> p m h e", h=NH)
                nc.gpsimd.memset(vv[:, :, :, D:D + 1], 1.0)
                nc.gpsimd.memset(vv[:, :, :, D + 1:HS], 0.0)
                for mb in range(8):
                    pa = ps_a.tile([128, 512], F32, tag="pa", name="pa")
                    for kb in range(2):
                        nc.tensor.matmul(
                            pa[:, 0:C],
                            rt[s][kb][:, mb * 128:(mb + 1) * 128],
                            wvt[:, kb, :], start=(kb == 0), stop=(kb == 1))
                    nc.vector.tensor_scalar(
                        vv[:, mb, :, 0:D],
                        pa[:, 0:C].rearrange("p (h d) -> p h d", h=NH),
                        0.0, None, mybir.AluOpType.add)

            def phase_a(s):
                a_conv(s)
                a_q(s)
                a_k(s)
                a_vt(s)

            def phase_b(s, extras=()):
                ytmp[s] = [yp.tile([128, N], BF16, tag=f"yt{s}{pb}",
                                   name=f"yt{s}{pb}")
                           for pb in range(2)]
                et_all = [[None] * 4 for _ in range(NH)]  # per (h, mb-pair)

                def emit_se(h, mb):
                    # scores^T for key block mb: out [128 keys, 1024 queries]
                    pb, ro = h // 2, (h % 2) * 64
                    dsl = slice(ro, ro + 64)
                    psc = ps_sc.tile([128, 1024], F32, tag="psc", name="psc")
                    for nch in range(2):
                        nc.tensor.matmul(
                            psc[:, nch * 512:(nch + 1) * 512],
                            kt[s][pb][dsl, mb * 128:(mb + 1) * 128],
                            qt[s][pb][dsl, nch * 512:(nch + 1) * 512],
                            start=True, stop=True)
                    pr, half = mb // 2, mb % 2
                    if half == 0:
                        et_all[h][pr] = ep.tile([128, 2048], FP8, tag="et",
                                                name="et")
                    dst = et_all[h][pr][:, half * 1024:(half + 1) * 1024]
                    if mb in DVE_MBS:
                        nc.vector._custom_dve(EXP32Q_ANT, out=dst, in0=psc,
                                              s0=EXP_C0, s1=EXP_C1)
                    else:
                        nc.scalar.activation(
                            out=dst, in_=psc,
                            func=mybir.ActivationFunctionType.Exp,
                            scale=SCALE_ACT, bias=nbias)

                def emit_pv(h):
                    pb, ro = h // 2, (h % 2) * 64
                    pat = ps_at.tile([HS, N], F32, tag="pat", name="pat")
                    for nch in range(2):
                        for pr in range(4):
                            nc.tensor.matmul(
                                pat[:, nch * 512:(nch + 1) * 512],
                                vt[s][:, 2 * pr:2 * pr + 2,
                                      h * HS:(h + 1) * HS],
                                et_all[h][pr].rearrange(
                                    "p (t n) -> p t n", t=2)[
                                    :, :, nch * 512:(nch + 1) * 512],
                                start=(pr == 0), stop=(pr == 3),
                                perf_mode=DR)
                    # normalize: recip of sumexp row (read straight from
                    # PSUM), broadcast, multiply
                    srow = sp.tile([1, N], F32, tag="srow", name="srow", bufs=1)
                    rr = sp.tile([1, N], F32, tag="rr", name="rr", bufs=1)
                    rb = sp.tile([64, N], F32, tag="rb", name="rb", bufs=1)
                    nc.scalar.copy(srow, pat[D:D + 1, :])
                    nc.vector.reciprocal_approx_fast(out=rr, in_=srow)
                    nc.gpsimd.partition_broadcast(rb, rr, channels=64)
                    nc.vector.tensor_tensor(
                        out=ytmp[s][pb][ro:ro + 64, :],
                        in0=pat[0:D, :], in1=rb, op=mybir.AluOpType.mult)

                PF = 6
                for h in range(NH):
                    for mb in (range(PF, 8) if h > 0 else range(8)):
                        emit_se(h, mb)
                    if h + 1 < NH:
                        for mb in range(PF):
                            emit_se(h + 1, mb)
                    emit_pv(h)
                    if h % 2 == 1:
                        # y = attn + r for finished pb (both bf16 -> DVE 2x)
                        pb = h // 2
                        nc.vector.tensor_tensor(
                            out=ytmp[s][pb], in0=ytmp[s][pb], in1=rt[s][pb],
                            op=mybir.AluOpType.add)
                    if extras and h < len(extras):
                        extras[h]()

            def c_conv(s):
                # conv2 + bn2 -> z (fp32) on ACT, with per-channel sums for
                # CBAM avg via accum_out
                zt = [zp.tile([128, N], F32, tag=f"z{ob}", name=f"z{ob}")
                      for ob in range(2)]
                cols = [zp.tile([128, 2], F32, tag=f"cols{ob}", name=f"cols{ob}")
                        for ob in range(2)]
                accs = [zp.tile([128, 2], F32, tag=f"acc{ob}", name=f"acc{ob}")
                        for ob in range(2)]
                for ob in range(2):
                    for nch in range(2):
                        pa = ps_a.tile([128, 512], F32, tag="pa", name="pa")
                        for kb in range(2):
                            nc.tensor.matmul(
                                pa, w2t[:, kb, ob * 128:(ob + 1) * 128],
                                ytmp[s][kb][:, nch * 512:(nch + 1) * 512],
                                start=(kb == 0), stop=(kb == 1))
                        if s == 1:
                            nc.scalar.activation(
                                out=zt[ob][:, nch * 512:(nch + 1) * 512],
                                in_=pa, bias=sh2[:, ob:ob + 1], scale=1.0,
                                func=mybir.ActivationFunctionType.Identity,
                                accum_out=accs[ob][:, nch:nch + 1])
                        else:
                            nc.vector.tensor_scalar(
                                zt[ob][:, nch * 512:(nch + 1) * 512], pa,
                                sh2[:, ob:ob + 1], 0.0,
                                mybir.AluOpType.add, mybir.AluOpType.add,
                                accum_out=accs[ob][:, nch:nch + 1])
                    # combine the two 512-chunk sums into col 0
                    nc.vector.tensor_tensor(
                        out=cols[ob][:, 0:1], in0=accs[ob][:, 0:1],
                        in1=accs[ob][:, 1:2], op=mybir.AluOpType.add)
                    # channel max per nch chunk (starts right after each
                    # z-add chunk), combined into col 1
                    mx = zp.tile([128, 2], F32, tag=f"mx{ob}", name=f"mx{ob}")
                    for nch in range(2):
                        nc.vector.tensor_reduce(
                            out=mx[:, nch:nch + 1],
                            in_=zt[ob][:, nch * 512:(nch + 1) * 512],
                            op=mybir.AluOpType.max,
                            axis=mybir.AxisListType.X)
                    nc.vector.tensor_tensor(
                        out=cols[ob][:, 1:2], in0=mx[:, 0:1],
                        in1=mx[:, 1:2], op=mybir.AluOpType.max)
                return zt, cols

            def c_chan(s, zt, cols):
                # channel attention: one fused FC1 matmul pair gives
                # [W1a@sum | W1m@max] stacked on 64 partitions; FC2 uses a
                # host-duplicated cw2 so both halves contract in place
                ph = ps_a.tile([128, 512], F32, tag="pa", name="pa")
                for kb in range(2):
                    nc.tensor.matmul(ph[0:2 * R, 0:2], cw1[:, kb, :],
                                     cols[kb][:, 0:2],
                                     start=(kb == 0), stop=(kb == 1))
                hsb = sp.tile([2 * R, 2], F32, tag="hsb", name="hsb")
                nc.vector.tensor_scalar(hsb, ph[0:2 * R, 0:2], 0.0, None,
                                        mybir.AluOpType.max)
                ca = [sp.tile([128, 1], F32, tag=f"ca{ob}", name=f"ca{ob}")
                      for ob in range(2)]
                for ob in range(2):
                    pc = ps_a.tile([128, 512], F32, tag="pa", name="pa")
                    nc.tensor.matmul(pc[:, 0:1],
                                     cw2[0:R, ob * 128:(ob + 1) * 128],
                                     hsb[0:R, 0:1], start=True, stop=False)
                    nc.tensor.matmul(pc[:, 0:1],
                                     cw2[R:2 * R, ob * 128:(ob + 1) * 128],
                                     hsb[R:2 * R, 1:2], start=False, stop=True)
                    nc.scalar.activation(
                        out=ca[ob], in_=pc[:, 0:1],
                        func=mybir.ActivationFunctionType.Tanh, scale=0.5)
                    nc.vector.tensor_scalar(
                        ca[ob], ca[ob], 0.5, 0.5,
                        mybir.AluOpType.mult, mybir.AluOpType.add)
                # apply channel attention -> z_ca (bf16)
                zca = [zp.tile([128, N], BF16, tag=f"zca{ob}", name=f"zca{ob}")
                       for ob in range(2)]
                for ob in range(2):
                    if s == 1:
                        nc.scalar.activation(
                            out=zca[ob], in_=zt[ob],
                            func=mybir.ActivationFunctionType.Copy,
                            scale=ca[ob])
                    else:
                        nc.vector.tensor_scalar_mul(zca[ob], zt[ob], ca[ob])
                return zca

            def c_spat(s, zca):
                # spatial sum (avg path) via ones matmul (bf16)
                avg_row = sp.tile([1, N], BF16, tag="avg_row", name="avg_row",
                                  bufs=1)
                for nch in range(2):
                    psr = ps_a.tile([128, 512], F32, tag="pa", name="pa")
                    for kb in range(2):
                        nc.tensor.matmul(
                            psr[0:1, :],
                            ones_fr, zca[kb][:, nch * 512:(nch + 1) * 512],
                            start=(kb == 0), stop=(kb == 1))
                    nc.scalar.copy(
                        avg_row[:, nch * 512:(nch + 1) * 512], psr[0:1, :])
                # spatial max via TT max (bf16 2x) + gpsimd reduce
                m1 = zp.tile([128, N], BF16, tag="m1", name="m1", bufs=1)
                nc.vector.tensor_tensor(out=m1, in0=zca[0], in1=zca[1],
                                        op=mybir.AluOpType.max)
                rep = zp.tile([128, N], BF16, tag="rep", name="rep", bufs=1)
                nc.gpsimd.partition_all_reduce(rep, m1, channels=128,
                                               reduce_op=bass_isa.ReduceOp.max)
                # reshape rows [1, 1024] -> [32(y), 32(x)] via SBUF->SBUF DMA
                avgT = sp.tile([32, 38], BF16, tag="avgT", name="avgT")
                nc.gpsimd.memset(avgT, 0.0)
                nc.sync.dma_start(out=avgT[:, 3:35], in_=avg_row)
                maxT = sp.tile([32, 38], BF16, tag="maxT", name="maxT")
                nc.gpsimd.memset(maxT, 0.0)
                nc.sync.dma_start(out=maxT[:, 3:35], in_=rep[0:1, :])
                # 7x7 conv as 14 banded matmuls over y, x-shifts on free dim
                psa = ps_a.tile([128, 512], F32, tag="pa", name="pa")
                first = True
                for c2, inp in ((0, avgT), (1, maxT)):
                    for kx in range(7):
                        nc.tensor.matmul(
                            psa[0:32, 0:32],
                            wbd[:, c2 * 7 + kx, :],
                            inp[:, kx:kx + 32],
                            start=first, stop=(c2 == 1 and kx == 6))
                        first = False
                sasb = sp.tile([32, 32], BF16, tag="sasb", name="sasb")
                nc.scalar.activation(
                    out=sasb, in_=psa[0:32, 0:32],
                    func=mybir.ActivationFunctionType.Tanh, scale=0.5)
                nc.vector.tensor_scalar(
                    sasb, sasb, 0.5, 0.5,
                    mybir.AluOpType.mult, mybir.AluOpType.add)
                sarow = sp.tile([1, N], BF16, tag="sarow", name="sarow")
                nc.sync.dma_start(out=sarow, in_=sasb)
                sarep = zp.tile([128, N], BF16, tag="sarep", name="sarep",
                                bufs=1)
                nc.gpsimd.partition_broadcast(sarep, sarow, channels=128)
                return sarep

            def c_fin(s, zca, sarep):
                # final: out = relu(z_ca * sa + x); cb0 TTs on DVE, cb1 TTs
                # on Pool so both channel blocks run in parallel in the tail
                us = [op_.tile([128, N], BF16, tag=f"fin{cb}", name=f"fin{cb}")
                      for cb in range(2)]
                os_ = [op_.tile([128, N], F32, tag=f"fino{cb}",
                                name=f"fino{cb}") for cb in range(2)]
                for ch in range(2):
                    lo, hi = ch * 512, ch * 512 + 512
                    for cb in range(2):
                        eng = nc.gpsimd if (cb == 1 and ch == 1) else nc.vector
                        u, o = us[cb], os_[cb]
                        eng.tensor_tensor(
                            out=u[:, lo:hi], in0=zca[cb][:, lo:hi],
                            in1=sarep[:, lo:hi], op=mybir.AluOpType.mult)
                        eng.tensor_tensor(
                            out=u[:, lo:hi], in0=u[:, lo:hi],
                            in1=xt[s][cb][:, lo:hi], op=mybir.AluOpType.add)
                        nc.vector.tensor_scalar(
                            o[:, lo:hi], u[:, lo:hi], 0.0, None,
                            mybir.AluOpType.max)
                        nc.sync.dma_start(out=out[s, cb][:, lo:hi],
                                          in_=o[:, lo:hi])

            def phase_c(s):
                zt, cols = c_conv(s)
                zca = c_chan(s, zt, cols)
                sarep = c_spat(s, zca)
                c_fin(s, zca, sarep)

            phase_a(0)
            phase_b(0, extras=(lambda: a_conv(1), lambda: a_q(1),
                               lambda: a_k(1), lambda: a_vt(1)))
            c0_state = {}

            def x0():
                c0_state["zt"], c0_state["cols"] = c_conv(0)

            def x1():
                c0_state["zca"] = c_chan(0, c0_state["zt"], c0_state["cols"])

            def x2():
                c0_state["sarep"] = c_spat(0, c0_state["zca"])

            def x3():
                c_fin(0, c0_state["zca"], c0_state["sarep"])

            phase_b(1, extras=(x0, x1, x2, x3))
            phase_c(1)

    nc.compile()
    return nc


_NC_CACHE = None


def get_module():
    global _NC_CACHE
    if _NC_CACHE is None:
        _NC_CACHE = build_module()
    return _NC_CACHE


def prep_inputs(x, w1, bn1_g, bn1_b, bn1_m, bn1_v, wq, bq, wk, bk, wv, bv,
                gamma, w2, bn2_g, bn2_b, bn2_m, bn2_v, ca_w1, ca_w2, sa_w):
    """Host-side preprocessing -> per-core in_maps."""
    f64 = np.float64
    bf16 = ml_dtypes.bfloat16
    s1 = (bn1_g.astype(f64) / np.sqrt(bn1_v.astype(f64) + EPS))
    w1f = (s1[:, None] * w1.astype(f64)).astype(np.float32)
    sh1 = (bn1_b.astype(f64) - bn1_m.astype(f64) * s1).astype(np.float32)
    s2 = (bn2_g.astype(f64) / np.sqrt(bn2_v.astype(f64) + EPS))
    w2f = (s2[:, None] * w2.astype(f64)).astype(np.float32)
    g = float(gamma[0])
    wvg = (wv.astype(f64) * g).astype(np.float32)
    bvg = (bv.astype(f64) * g)
    # v-bias folds into the conv2 shift: softmax rows sum to 1, so the
    # attention output gets exactly +bvg per channel before conv2.
    sh2 = (bn2_b.astype(f64) - bn2_m.astype(f64) * s2
           + (s2[:, None] * w2.astype(f64)) @ bvg).astype(np.float32)

    def lhsT(w):  # [O, C] -> [2, 128, O] kb-blocked transpose, bf16
        return np.ascontiguousarray(w.T.reshape(2, 128, C)).astype(bf16)

    def lhsT_qk(w):
        wt = (w.astype(f64) * ALPHA).astype(np.float32).T  # [C_in, O]
        return np.ascontiguousarray(wt.reshape(2, 128, C)).astype(bf16)

    def bias_qk(b):
        bb = (b.astype(f64) * ALPHA).astype(np.float32)
        return np.ascontiguousarray(bb.reshape(2, 128, 1))

    base = {
        "w1fT": lhsT(w1f),
        "wqT": lhsT_qk(wq),
        "wkT": lhsT_qk(wk),
        "wvT": np.ascontiguousarray(wvg.T.reshape(2, 128, C)).astype(bf16),
        "w2fT": lhsT(w2f),
        "shift1": np.ascontiguousarray(sh1.reshape(2, 128, 1)),
        "bq_c": bias_qk(bq),
        "shift2": np.ascontiguousarray(sh2.reshape(2, 128, 1)),
    }
    # channel attention weights: caw1T [2, 128, 64]
    c1T = ca_w1.T.astype(np.float32)             # [C, R]
    caw1T = np.concatenate([c1T / float(N), c1T], axis=1)  # [C, 2R]
    base["caw1T"] = np.ascontiguousarray(caw1T.reshape(2, 128, 2 * R))
    c2T = ca_w2.T.astype(np.float32)             # [R, C]
    base["caw2T"] = np.ascontiguousarray(np.concatenate([c2T, c2T], axis=0))
    # spatial conv bands: wband[yi, c2*7+kx, yo] = w[c2, yi-yo+3, kx]
    wb = np.zeros((32, 14, 32), np.float32)
    for c2 in range(2):
        for kx in range(7):
            for yo in range(32):
                for ky in range(7):
                    yi = yo + ky - 3
                    if 0 <= yi < 32:
                        v = sa_w[0, c2, ky, kx]
                        if c2 == 0:
                            v = v / float(C)
                        wb[yi, c2 * 7 + kx, yo] = v
    base["wband"] = wb.astype(bf16)
    base["ones_in"] = np.ones((128, 1), bf16)

    xrf = x.reshape(B, C, N).astype(bf16)
    in_maps = []
    for core in range(NCORES):
        m = dict(base)
        m["xr"] = np.ascontiguousarray(
            xrf[core * SPC:(core + 1) * SPC].reshape(SPC, 2, 128, N))
        in_maps.append(m)
    return in_maps


def kernel(**inputs):
    nc = get_module()
    in_maps = prep_inputs(**inputs)
    res = run_bass_kernel_spmd(nc, in_maps, core_ids=list(range(NCORES)))
    outs = []
    for core in range(NCORES):
        o = res.results[core]["out"]  # [SPC, 2, 128, N]
        outs.append(o.reshape(SPC, C, H, W))
    return np.concatenate(outs, axis=0)


if __name__ == "__main__":
    nc = get_module()
    print("compiled ok")


# revision 18
# speedup vs baseline: 1.0224x; 1.0005x over previous
"""Trainium2 Bass kernel for nn_BottleneckTransformer.

Data-parallel over batch: B=16 samples -> 8 cores x 2 samples.
Key optimizations over the f32r/bf16 baseline:
  - scores^T and PV matmuls in fp8e4 with DoubleRow perf mode; q/k stored
    plane-packed, v^T stored with 80-wide head slots.
  - softmax exp split across ACT (table exp) and DVE (custom EXP32Q_ANT,
    p=(m^2+c)^32 ~ e^(s-1.2)); the e^-1.2 shift avoids fp8e4 overflow and
    cancels in the sumexp normalization.
  - k-bias dropped (constant per query -> cancels in softmax); v-bias folded
    into the conv2 shift on the host (softmax rows sum to 1).
  - sumexp reciprocal reads PSUM directly (no ACT-side row copy).
  - x, conv1 weights, CBAM spatial chain in bf16; final fuse on DVE with
    bf16 2x modes; conv2 z-add + channel sums on ACT via accum_out.
"""
import numpy as np
import ml_dtypes

import concourse.bacc as bacc
import concourse.bass as bass
import concourse.tile as tile
from concourse import mybir, bass_isa
from concourse.bass_utils import run_bass_kernel_spmd

# ---- custom DVE op: p = ((x + c0)^2 + c1)^32 -------------------------------
from concourse.dve_spec import Spec, Src0, C0, C1, Bin, AluOp, lower as _dve_lower
from concourse.dve_uop import DveOpSpec
import concourse.dve_ops as _dve_ops_mod
from concourse.dve_ops import DveOp as _DveOp


def _ref_exp32q(in0, in1, c0, c1, c2):
    f32 = np.float32
    t = in0.astype(f32)
    c0a = np.asarray(c0, f32)
    c1a = np.asarray(c1, f32)
    m = (t + c0a).astype(f32)
    p = (m * m).astype(f32)
    p = (p + c1a).astype(f32)
    for _ in range(5):
        p = (p * p).astype(f32)
    return p


def _register_exp32q():
    if "EXP32Q_ANT" in _dve_ops_mod.CUSTOM_DVE_SPECS:
        for op in _dve_ops_mod.OPS:
            if op.name == "EXP32Q_ANT":
                return op
    m = Src0 + C0
    p = Bin(AluOp.MULTIPLY, m, m) + C1
    for _ in range(5):
        p = Bin(AluOp.MULTIPLY, p, p)
    spec = Spec(body=p, reference=_ref_exp32q)
    row = _dve_ops_mod._CUSTOM_DVE_ROW_BASE + len(_dve_ops_mod.OPS)
    shas = {}
    for ver in ("v3", "v4"):
        uops = _dve_lower(spec, ver=ver)
        shas[ver] = DveOpSpec(name="EXP32Q_ANT", opcode=row, uops=uops,
                              rd1_en=False).sha(ver)
    op = _DveOp("EXP32Q_ANT", spec, subdim=False, uops_sha=shas)
    _dve_ops_mod.OPS.append(op)
    _dve_ops_mod.CUSTOM_DVE_SPECS[op.name] = op.spec
    _dve_ops_mod._SUB_OPCODE_FOR_NAME[op.name] = row
    return op


EXP32Q_ANT = _register_exp32q()

F32 = mybir.dt.float32
F32R = mybir.dt.float32r
BF16 = mybir.dt.bfloat16
FP8 = mybir.dt.float8e4

B, C, H, W = 16, 256, 32, 32
N = H * W          # 1024
NCORES = 8
SPC = B // NCORES  # samples per core = 2
NH, D = 4, 64      # heads, head dim
HS = 80            # padded head slot in v^T (16B-aligned stationary stride)
R = C // 8         # 32, channel attention bottleneck
EPS = 1e-5
DR = mybir.MatmulPerfMode.DoubleRow

# exp scaling: scores arrive as x = A2*(q.k) with A2 = sqc*ga/256;
# DVE computes ((x+C0)^2+C1)^32 = e^(qk/8 - 1.2); ACT uses
# exp(SCALE_ACT*x - 1.2).  The e^-1.2 shift cancels in normalization.
SHIFT = 1.2
SQC = float(np.exp(-SHIFT / 64.0))        # c^(1/2) with c = e^(-SHIFT/32)
GA = float(np.sqrt(2.0) / 2.0)
EXP_C0 = float(np.float32(SQC * GA))
EXP_C1 = float(np.float32(0.5 * np.exp(-SHIFT / 32.0)))
A2 = SQC * GA / 256.0                     # per-pair scale on q.k
ALPHA = float(np.sqrt(A2))                # fold into each of wq,bq,wk
SCALE_ACT = float(np.float32(32.0 / (SQC * GA)))

import os
# key blocks (mb) whose exp runs on the DVE custom op; rest on ACT
DVE_MBS = set(int(c) for c in os.environ.get("KNOB_DVE_MBS", "357"))


def build_module():
    nc = bacc.Bacc("TRN2", target_bir_lowering=False, debug=False)

    def din(name, shape, dt=F32):
        return nc.dram_tensor(name, shape, dt, kind="ExternalInput").ap()

    def dout(name, shape, dt=F32):
        return nc.dram_tensor(name, shape, dt, kind="ExternalOutput").ap()

    xr = din("xr", (SPC, 2, 128, N), BF16)        # per-sample x, c-blocks
    w1fT = din("w1fT", (2, 128, C), BF16)         # [kb][c,128 -> o cols]
    wqT = din("wqT", (2, 128, C), BF16)           # ALPHA-scaled
    wkT = din("wkT", (2, 128, C), BF16)
    wvT = din("wvT", (2, 128, C), BF16)           # gamma folded
    w2fT = din("w2fT", (2, 128, C), BF16)
    shift1 = din("shift1", (2, 128, 1), F32)
    bq_c = din("bq_c", (2, 128, 1), F32)          # per-plane bias (scaled)
    shift2 = din("shift2", (2, 128, 1), F32)      # bn2 shift + w2f@bvg
    caw1T = din("caw1T", (2, 128, 2 * R), F32)    # cols 0:32 avg(/1024), 32:64 max
    caw2T = din("caw2T", (2 * R, C), F32)
    wband = din("wband", (32, 14, 32), BF16)      # sa conv bands, (c2,kx)
    ones_in = din("ones_in", (128, 1), BF16)

    out = dout("out", (SPC, 2, 128, N), F32)
    scr_sa = dout("scr_sa", (SPC, 32, 32), BF16)  # bounce: sigmoid(sa)

    with tile.TileContext(nc) as tc:
        with (
            tc.tile_pool(name="wpool", bufs=1) as wp,
            tc.tile_pool(name="xpool", bufs=1) as xp,
            tc.tile_pool(name="rpool", bufs=1) as rp,
            tc.tile_pool(name="qkpool", bufs=1) as qkp,
            tc.tile_pool(name="vpool", bufs=1) as vp,
            tc.tile_pool(name="epool", bufs=13) as ep,
            tc.tile_pool(name="ypool", bufs=1) as yp,
            tc.tile_pool(name="zpool", bufs=2) as zp,
            tc.tile_pool(name="spool", bufs=2) as sp,
            tc.tile_pool(name="opool", bufs=2) as op_,
            tc.tile_pool(name="ps_sc", bufs=2, space="PSUM") as ps_sc,
            tc.tile_pool(name="ps_at", bufs=1, space="PSUM") as ps_at,
            tc.tile_pool(name="ps_a", bufs=2, space="PSUM") as ps_a,
        ):
            # ---- load order: conv1 weights + inputs first ----
            w1t = wp.tile([128, 2, C], BF16, tag="w1t", name="w1t")
            nc.sync.dma_start(out=w1t, in_=w1fT.rearrange("k p c -> p k c"))
            sh1 = wp.tile([128, 2], F32, tag="sh1", name="sh1")
            nc.sync.dma_start(out=sh1, in_=shift1.rearrange("k p a -> p (k a)"))
            xt_all = [[xp.tile([128, N], BF16, tag=f"x{si}{cb}", name=f"x{si}{cb}")
                       for cb in range(2)] for si in range(SPC)]
            for nch in range(2):
                for cb in range(2):
                    nc.sync.dma_start(
                        out=xt_all[0][cb][:, nch * 512:(nch + 1) * 512],
                        in_=xr[0, cb][:, nch * 512:(nch + 1) * 512])
            wqt = wp.tile([128, 2, C], BF16, tag="wqt", name="wqt")
            nc.sync.dma_start(out=wqt, in_=wqT.rearrange("k p c -> p k c"))
            wkt = wp.tile([128, 2, C], BF16, tag="wkt", name="wkt")
            nc.sync.dma_start(out=wkt, in_=wkT.rearrange("k p c -> p k c"))
            for cb in range(2):
                nc.sync.dma_start(out=xt_all[1][cb], in_=xr[1, cb])
            bqc = wp.tile([128, 2], F32, tag="bqc", name="bqc")
            nc.sync.dma_start(out=bqc, in_=bq_c.rearrange("k p a -> p (k a)"))
            wvt = wp.tile([128, 2, C], BF16, tag="wvt", name="wvt")
            nc.sync.dma_start(out=wvt, in_=wvT.rearrange("k p c -> p k c"))
            w2t = wp.tile([128, 2, C], BF16, tag="w2t", name="w2t")
            nc.sync.dma_start(out=w2t, in_=w2fT.rearrange("k p c -> p k c"))
            sh2 = wp.tile([128, 2], F32, tag="sh2", name="sh2")
            nc.sync.dma_start(out=sh2, in_=shift2.rearrange("k p a -> p (k a)"))
            cw1 = wp.tile([128, 2, 2 * R], F32, tag="cw1", name="cw1")
            nc.sync.dma_start(out=cw1, in_=caw1T.rearrange("k p c -> p k c"))
            cw2 = wp.tile([2 * R, C], F32, tag="cw2", name="cw2")
            nc.sync.dma_start(out=cw2, in_=caw2T)
            wbd = wp.tile([32, 14, 32], BF16, tag="wbd", name="wbd")
            nc.sync.dma_start(out=wbd, in_=wband)
            ones_fr = wp.tile([128, 1], BF16, tag="ones_fr", name="ones_fr")
            nc.sync.dma_start(out=ones_fr, in_=ones_in)
            nbias = wp.tile([128, 1], F32, tag="nbias", name="nbias")
            nc.vector.memset(nbias, -SHIFT)

            xt = [None] * SPC      # [s][cb] bf16 input tiles
            rt = [None] * SPC      # relu(conv1) tiles (bf16)
            qt = [None] * SPC      # [128, N] fp8 per ob
            kt = [None] * SPC
            vt = [None] * SPC      # vT fp8 [128, mb, NH*HS]
            ytmp = [None] * SPC    # y = attn + r (bf16)

            def a_conv(s):
                xt[s] = xt_all[s]
                rt[s] = [rp.tile([128, N], BF16, tag=f"r{s}{ob}", name=f"r{s}{ob}")
                         for ob in range(2)]
                for ob in range(2):
                    for nch in range(2):
                        pa = ps_a.tile([128, 512], F32, tag="pa", name="pa")
                        for kb in range(2):
                            nc.tensor.matmul(
                                pa, w1t[:, kb, ob * 128:(ob + 1) * 128],
                                xt[s][kb][:, nch * 512:(nch + 1) * 512],
                                start=(kb == 0), stop=(kb == 1))
                        nc.scalar.activation(
                            out=rt[s][ob][:, nch * 512:(nch + 1) * 512],
                            in_=pa, bias=sh1[:, ob:ob + 1], scale=1.0,
                            func=mybir.ActivationFunctionType.Relu)

            def a_qk(s, dst, wt, bc, on_act):
                for ob in range(2):
                    for nch in range(2):
                        pa = ps_a.tile([128, 512], F32, tag="pa", name="pa")
                        for kb in range(2):
                            nc.tensor.matmul(
                                pa, wt[:, kb, ob * 128:(ob + 1) * 128],
                                rt[s][kb][:, nch * 512:(nch + 1) * 512],
                                start=(kb == 0), stop=(kb == 1))
                        if on_act:
                            nc.scalar.activation(
                                out=dst[ob][:, nch * 512:(nch + 1) * 512],
                                in_=pa,
                                bias=(bc[:, ob:ob + 1] if bc is not None
                                      else 0.0),
                                scale=1.0,
                                func=(mybir.ActivationFunctionType.Identity
                                      if bc is not None else
                                      mybir.ActivationFunctionType.Copy))
                        else:
                            nc.vector.tensor_scalar(
                                dst[ob][:, nch * 512:(nch + 1) * 512], pa,
                                0.0, None, mybir.AluOpType.add)

            def a_q(s):
                qt[s] = [qkp.tile([128, N], FP8, tag=f"q{s}{ob}",
                                  name=f"q{s}{ob}") for ob in range(2)]
                a_qk(s, qt[s], wqt, bqc, True)

            def a_k(s):
                kt[s] = [qkp.tile([128, N], FP8, tag=f"k{s}{ob}",
                                  name=f"k{s}{ob}") for ob in range(2)]
                a_qk(s, kt[s], wkt, None, False)

            def a_vt(s):
                vt[s] = vp.tile([128, 8, NH * HS], FP8, tag=f"v{s}", name=f"v{s}")
                vv = vt[s].rearrange("p m (h e) -> p m h e", h=NH)
                nc.gpsimd.memset(vv[:, :, :, D:D + 1], 1.0)
                nc.gpsimd.memset(vv[:, :, :, D + 1:HS], 0.0)
                for mb in range(8):
                    pa = ps_a.tile([128, 512], F32, tag="pa", name="pa")
                    for kb in range(2):
                        nc.tensor.matmul(
                            pa[:, 0:C],
                            rt[s][kb][:, mb * 128:(mb + 1) * 128],
                            wvt[:, kb, :], start=(kb == 0), stop=(kb == 1))
                    nc.vector.tensor_scalar(
                        vv[:, mb, :, 0:D],
                        pa[:, 0:C].rearrange("p (h d) -> p h d", h=NH),
                        0.0, None, mybir.AluOpType.add)

            def phase_a(s):
                a_conv(s)
                a_q(s)
                a_k(s)
                a_vt(s)

            def phase_b(s, extras=()):
                ytmp[s] = [yp.tile([128, N], BF16, tag=f"yt{s}{pb}",
                                   name=f"yt{s}{pb}")
                           for pb in range(2)]
                et_all = [[None] * 4 for _ in range(NH)]  # per (h, mb-pair)

                def emit_se(h, mb):
                    # scores^T for key block mb: out [128 keys, 1024 queries]
                    pb, ro = h // 2, (h % 2) * 64
                    dsl = slice(ro, ro + 64)
                    psc = ps_sc.tile([128, 1024], F32, tag="psc", name="psc")
                    for nch in range(2):
                        nc.tensor.matmul(
                            psc[:, nch * 512:(nch + 1) * 512],
                            kt[s][pb][dsl, mb * 128:(mb + 1) * 128],
                            qt[s][pb][dsl, nch * 512:(nch + 1) * 512],
                            start=True, stop=True)
                    pr, half = mb // 2, mb % 2
                    if half == 0:
                        et_all[h][pr] = ep.tile([128, 2048], FP8, tag="et",
                                                name="et")
                    dst = et_all[h][pr][:, half * 1024:(half + 1) * 1024]
                    if mb in DVE_MBS:
                        nc.vector._custom_dve(EXP32Q_ANT, out=dst, in0=psc,
                                              s0=EXP_C0, s1=EXP_C1)
                    else:
                        nc.scalar.activation(
                            out=dst, in_=psc,
                            func=mybir.ActivationFunctionType.Exp,
                            scale=SCALE_ACT, bias=nbias)

                def emit_pv(h):
                    pb, ro = h // 2, (h % 2) * 64
                    pat = ps_at.tile([HS, N], F32, tag="pat", name="pat")
                    for nch in range(2):
                        for pr in range(4):
                            nc.tensor.matmul(
                                pat[:, nch * 512:(nch + 1) * 512],
                                vt[s][:, 2 * pr:2 * pr + 2,
                                      h * HS:(h + 1) * HS],
                                et_all[h][pr].rearrange(
                                    "p (t n) -> p t n", t=2)[
                                    :, :, nch * 512:(nch + 1) * 512],
                                start=(pr == 0), stop=(pr == 3),
                                perf_mode=DR)
                    # normalize: recip of sumexp row (read straight from
                    # PSUM), broadcast, multiply
                    srow = sp.tile([1, N], F32, tag="srow", name="srow", bufs=1)
                    rr = sp.tile([1, N], F32, tag="rr", name="rr", bufs=1)
                    rb = sp.tile([64, N], F32, tag="rb", name="rb", bufs=1)
                    nc.scalar.copy(srow, pat[D:D + 1, :])
                    nc.vector.reciprocal_approx_fast(out=rr, in_=srow)
                    nc.gpsimd.partition_broadcast(rb, rr, channels=64)
                    nc.vector.tensor_tensor(
                        out=ytmp[s][pb][ro:ro + 64, :],
                        in0=pat[0:D, :], in1=rb, op=mybir.AluOpType.mult)

                PF = 6
                for h in range(NH):
                    for mb in (range(PF, 8) if h > 0 else range(8)):
                        emit_se(h, mb)
                    if h + 1 < NH:
                        for mb in range(PF):
                            emit_se(h + 1, mb)
                    emit_pv(h)
                    if h % 2 == 1:
                        # y = attn + r for finished pb (both bf16 -> DVE 2x)
                        pb = h // 2
                        nc.vector.tensor_tensor(
                            out=ytmp[s][pb], in0=ytmp[s][pb], in1=rt[s][pb],
                            op=mybir.AluOpType.add)
                    if extras and h < len(extras):
                        extras[h]()

            def c_conv(s):
                # conv2 + bn2 -> z (fp32) on ACT, with per-channel sums for
                # CBAM avg via accum_out
                zt = [zp.tile([128, N], F32, tag=f"z{ob}", name=f"z{ob}")
                      for ob in range(2)]
                cols = [zp.tile([128, 2], F32, tag=f"cols{ob}", name=f"cols{ob}")
                        for ob in range(2)]
                accs = [zp.tile([128, 2], F32, tag=f"acc{ob}", name=f"acc{ob}")
                        for ob in range(2)]
                for ob in range(2):
                    for nch in range(2):
                        pa = ps_a.tile([128, 512], F32, tag="pa", name="pa")
                        for kb in range(2):
                            nc.tensor.matmul(
                                pa, w2t[:, kb, ob * 128:(ob + 1) * 128],
                                ytmp[s][kb][:, nch * 512:(nch + 1) * 512],
                                start=(kb == 0), stop=(kb == 1))
                        if s == 1:
                            nc.scalar.activation(
                                out=zt[ob][:, nch * 512:(nch + 1) * 512],
                                in_=pa, bias=sh2[:, ob:ob + 1], scale=1.0,
                                func=mybir.ActivationFunctionType.Identity,
                                accum_out=accs[ob][:, nch:nch + 1])
                        else:
                            nc.vector.tensor_scalar(
                                zt[ob][:, nch * 512:(nch + 1) * 512], pa,
                                sh2[:, ob:ob + 1], 0.0,
                                mybir.AluOpType.add, mybir.AluOpType.add,
                                accum_out=accs[ob][:, nch:nch + 1])
                    # combine the two 512-chunk sums into col 0
                    nc.vector.tensor_tensor(
                        out=cols[ob][:, 0:1], in0=accs[ob][:, 0:1],
                        in1=accs[ob][:, 1:2], op=mybir.AluOpType.add)
                    # channel max per nch chunk (starts right after each
                    # z-add chunk), combined into col 1
                    mx = zp.tile([128, 2], F32, tag=f"mx{ob}", name=f"mx{ob}")
                    for nch in range(2):
                        nc.vector.tensor_reduce(
                            out=mx[:, nch:nch + 1],
                            in_=zt[ob][:, nch * 512:(nch + 1) * 512],
                            op=mybir.AluOpType.max,
                            axis=mybir.AxisListType.X)
                    nc.vector.tensor_tensor(
                        out=cols[ob][:, 1:2], in0=mx[:, 0:1],
                        in1=mx[:, 1:2], op=mybir.AluOpType.max)
                return zt, cols

            def c_chan(s, zt, cols):
                # channel attention: one fused FC1 matmul pair gives
                # [W1a@sum | W1m@max] stacked on 64 partitions; FC2 uses a
                # host-duplicated cw2 so both halves contract in place
                ph = ps_a.tile([128, 512], F32, tag="pa", name="pa")
                for kb in range(2):
                    nc.tensor.matmul(ph[0:2 * R, 0:2], cw1[:, kb, :],
                                     cols[kb][:, 0:2],
                                     start=(kb == 0), stop=(kb == 1))
                hsb = sp.tile([2 * R, 2], F32, tag="hsb", name="hsb")
                nc.vector.tensor_scalar(hsb, ph[0:2 * R, 0:2], 0.0, None,
                                        mybir.AluOpType.max)
                ca = [sp.tile([128, 1], F32, tag=f"ca{ob}", name=f"ca{ob}")
                      for ob in range(2)]
                for ob in range(2):
                    pc = ps_a.tile([128, 512], F32, tag="pa", name="pa")
                    nc.tensor.matmul(pc[:, 0:1],
                                     cw2[0:R, ob * 128:(ob + 1) * 128],
                                     hsb[0:R, 0:1], start=True, stop=False)
                    nc.tensor.matmul(pc[:, 0:1],
                                     cw2[R:2 * R, ob * 128:(ob + 1) * 128],
                                     hsb[R:2 * R, 1:2], start=False, stop=True)
                    nc.scalar.activation(
                        out=ca[ob], in_=pc[:, 0:1],
                        func=mybir.ActivationFunctionType.Tanh, scale=0.5)
                    nc.vector.tensor_scalar(
                        ca[ob], ca[ob], 0.5, 0.5,
                        mybir.AluOpType.mult, mybir.AluOpType.add)
                # apply channel attention -> z_ca (bf16)
                zca = [zp.tile([128, N], BF16, tag=f"zca{ob}", name=f"zca{ob}")
                       for ob in range(2)]
                for ob in range(2):
                    if s == 1:
                        nc.scalar.activation(
                            out=zca[ob], in_=zt[ob],
                            func=mybir.ActivationFunctionType.Copy,
                            scale=ca[ob])
                    else:
                        nc.vector.tensor_scalar_mul(zca[ob], zt[ob], ca[ob])
                return zca

            def c_spat(s, zca):
                # spatial sum (avg path) via ones matmul (bf16)
                avg_row = sp.tile([1, N], BF16, tag="avg_row", name="avg_row",
                                  bufs=1)
                for nch in range(2):
                    psr = ps_a.tile([128, 512], F32, tag="pa", name="pa")
                    for kb in range(2):
                        nc.tensor.matmul(
                            psr[0:1, :],
                            ones_fr, zca[kb][:, nch * 512:(nch + 1) * 512],
                            start=(kb == 0), stop=(kb == 1))
                    nc.scalar.copy(
                        avg_row[:, nch * 512:(nch + 1) * 512], psr[0:1, :])
                # spatial max via TT max (bf16 2x) + gpsimd reduce
                m1 = zp.tile([128, N], BF16, tag="m1", name="m1", bufs=1)
                nc.vector.tensor_tensor(out=m1, in0=zca[0], in1=zca[1],
                                        op=mybir.AluOpType.max)
                rep = zp.tile([128, N], BF16, tag="rep", name="rep", bufs=1)
                nc.gpsimd.partition_all_reduce(rep, m1, channels=128,
                                               reduce_op=bass_isa.ReduceOp.max)
                # reshape rows [1, 1024] -> [32(y), 32(x)] via SBUF->SBUF DMA
                avgT = sp.tile([32, 38], BF16, tag="avgT", name="avgT")
                nc.gpsimd.memset(avgT, 0.0)
                nc.sync.dma_start(out=avgT[:, 3:35], in_=avg_row)
                maxT = sp.tile([32, 38], BF16, tag="maxT", name="maxT")
                nc.gpsimd.memset(maxT, 0.0)
                nc.sync.dma_start(out=maxT[:, 3:35], in_=rep[0:1, :])
                # 7x7 conv as 14 banded matmuls over y, x-shifts on free dim
                psa = ps_a.tile([128, 512], F32, tag="pa", name="pa")
                first = True
                for c2, inp in ((0, avgT), (1, maxT)):
                    for kx in range(7):
                        nc.tensor.matmul(
                            psa[0:32, 0:32],
                            wbd[:, c2 * 7 + kx, :],
                            inp[:, kx:kx + 32],
                            start=first, stop=(c2 == 1 and kx == 6))
                        first = False
                sasb = sp.tile([32, 32], BF16, tag="sasb", name="sasb")
                nc.scalar.activation(
                    out=sasb, in_=psa[0:32, 0:32],
                    func=mybir.ActivationFunctionType.Tanh, scale=0.5)
                nc.vector.tensor_scalar(
                    sasb, sasb, 0.5, 0.5,
                    mybir.AluOpType.mult, mybir.AluOpType.add)
                sarow = sp.tile([1, N], BF16, tag="sarow", name="sarow")
                nc.sync.dma_start(out=sarow, in_=sasb)
                sarep = zp.tile([128, N], BF16, tag="sarep", name="sarep",
                                bufs=1)
                nc.gpsimd.partition_broadcast(sarep, sarow, channels=128)
                return sarep

            def c_fin(s, zca, sarep):
                # final: out = relu(z_ca * sa + x); cb0 TTs on DVE, cb1 TTs
                # on Pool so both channel blocks run in parallel in the tail
                us = [op_.tile([128, N], BF16, tag=f"fin{cb}", name=f"fin{cb}")
                      for cb in range(2)]
                os_ = [op_.tile([128, N], F32, tag=f"fino{cb}",
                                name=f"fino{cb}") for cb in range(2)]
                for ch in range(2):
                    lo, hi = ch * 512, ch * 512 + 512
                    for cb in range(2):
                        eng = nc.gpsimd if (cb == 1 and ch == 1) else nc.vector
                        u, o = us[cb], os_[cb]
                        eng.tensor_tensor(
                            out=u[:, lo:hi], in0=zca[cb][:, lo:hi],
                            in1=sarep[:, lo:hi], op=mybir.AluOpType.mult)
                        eng.tensor_tensor(
                            out=u[:, lo:hi], in0=u[:, lo:hi],
                            in1=xt[s][cb][:, lo:hi], op=mybir.AluOpType.add)
                        nc.vector.tensor_scalar(
                            o[:, lo:hi], u[:, lo:hi], 0.0, None,
                            mybir.AluOpType.max)
                        nc.sync.dma_start(out=out[s, cb][:, lo:hi],
                                          in_=o[:, lo:hi])

            def phase_c(s):
                zt, cols = c_conv(s)
                zca = c_chan(s, zt, cols)
                sarep = c_spat(s, zca)
                c_fin(s, zca, sarep)

            phase_a(0)
            phase_b(0, extras=(lambda: a_conv(1), lambda: a_q(1),
                               lambda: a_k(1), lambda: a_vt(1)))
            c0_state = {}

            def x0():
                c0_state["zt"], c0_state["cols"] = c_conv(0)

            def x1():
                c0_state["zca"] = c_chan(0, c0_state["zt"], c0_state["cols"])

            def x2():
                c0_state["sarep"] = c_spat(0, c0_state["zca"])

            def x3():
                c_fin(0, c0_state["zca"], c0_state["sarep"])

            phase_b(1, extras=(x0, x1, x2, x3))
            phase_c(1)

    nc.compile()
    return nc


_NC_CACHE = None


def get_module():
    global _NC_CACHE
    if _NC_CACHE is None:
        _NC_CACHE = build_module()
    return _NC_CACHE


def prep_inputs(x, w1, bn1_g, bn1_b, bn1_m, bn1_v, wq, bq, wk, bk, wv, bv,
                gamma, w2, bn2_g, bn2_b, bn2_m, bn2_v, ca_w1, ca_w2, sa_w):
    """Host-side preprocessing -> per-core in_maps."""
    f64 = np.float64
    bf16 = ml_dtypes.bfloat16
    s1 = (bn1_g.astype(f64) / np.sqrt(bn1_v.astype(f64) + EPS))
    w1f = (s1[:, None] * w1.astype(f64)).astype(np.float32)
    sh1 = (bn1_b.astype(f64) - bn1_m.astype(f64) * s1).astype(np.float32)
    s2 = (bn2_g.astype(f64) / np.sqrt(bn2_v.astype(f64) + EPS))
    w2f = (s2[:, None] * w2.astype(f64)).astype(np.float32)
    g = float(gamma[0])
    wvg = (wv.astype(f64) * g).astype(np.float32)
    bvg = (bv.astype(f64) * g)
    # v-bias folds into the conv2 shift: softmax rows sum to 1, so the
    # attention output gets exactly +bvg per channel before conv2.
    sh2 = (bn2_b.astype(f64) - bn2_m.astype(f64) * s2
           + (s2[:, None] * w2.astype(f64)) @ bvg).astype(np.float32)

    def lhsT(w):  # [O, C] -> [2, 128, O] kb-blocked transpose, bf16
        return np.ascontiguousarray(w.T.reshape(2, 128, C)).astype(bf16)

    def lhsT_qk(w):
        wt = (w.astype(f64) * ALPHA).astype(np.float32).T  # [C_in, O]
        return np.ascontiguousarray(wt.reshape(2, 128, C)).astype(bf16)

    def bias_qk(b):
        bb = (b.astype(f64) * ALPHA).astype(np.float32)
        return np.ascontiguousarray(bb.reshape(2, 128, 1))

    base = {
        "w1fT": lhsT(w1f),
        "wqT": lhsT_qk(wq),
        "wkT": lhsT_qk(wk),
        "wvT": np.ascontiguousarray(wvg.T.reshape(2, 128, C)).astype(bf16),
        "w2fT": lhsT(w2f),
        "shift1": np.ascontiguousarray(sh1.reshape(2, 128, 1)),
        "bq_c": bias_qk(bq),
        "shift2": np.ascontiguousarray(sh2.reshape(2, 128, 1)),
    }
    # channel attention weights: caw1T [2, 128, 64]
    c1T = ca_w1.T.astype(np.float32)             # [C, R]
    caw1T = np.concatenate([c1T / float(N), c1T], axis=1)  # [C, 2R]
    base["caw1T"] = np.ascontiguousarray(caw1T.reshape(2, 128, 2 * R))
    c2T = ca_w2.T.astype(np.float32)             # [R, C]
    base["caw2T"] = np.ascontiguousarray(np.concatenate([c2T, c2T], axis=0))
    # spatial conv bands: wband[yi, c2*7+kx, yo] = w[c2, yi-yo+3, kx]
    wb = np.zeros((32, 14, 32), np.float32)
    for c2 in range(2):
        for kx in range(7):
            for yo in range(32):
                for ky in range(7):
                    yi = yo + ky - 3
                    if 0 <= yi < 32:
                        v = sa_w[0, c2, ky, kx]
                        if c2 == 0:
                            v = v / float(C)
                        wb[yi, c2 * 7 + kx, yo] = v
    base["wband"] = wb.astype(bf16)
    base["ones_in"] = np.ones((128, 1), bf16)

    xrf = x.reshape(B, C, N).astype(bf16)
    in_maps = []
    for core in range(NCORES):
        m = dict(base)
        m["xr"] = np.ascontiguousarray(
            xrf[core * SPC:(core + 1) * SPC].reshape(SPC, 2, 128, N))
        in_maps.append(m)
    return in_maps


def kernel(**inputs):
    nc = get_module()
    in_maps = prep_inputs(**inputs)
    res = run_bass_kernel_spmd(nc, in_maps, core_ids=list(range(NCORES)))
    outs = []
    for core in range(NCORES):
        o = res.results[core]["out"]  # [SPC, 2, 128, N]
        outs.append(o.reshape(SPC, C, H, W))
    return np.concatenate(outs, axis=0)


if __name__ == "__main__":
    nc = get_module()
    print("compiled ok")


# revision 19
# speedup vs baseline: 1.0380x; 1.0153x over previous
"""Trainium2 Bass kernel for nn_BottleneckTransformer.

Data-parallel over batch: B=16 samples -> 8 cores x 2 samples.
Key optimizations over the f32r/bf16 baseline:
  - scores^T and PV matmuls in fp8e4 with DoubleRow perf mode; q/k stored
    plane-packed, v^T stored with 80-wide head slots.
  - softmax exp split across ACT (table exp) and DVE (custom EXP32Q_ANT,
    p=(m^2+c)^32 ~ e^(s-1.2)); the e^-1.2 shift avoids fp8e4 overflow and
    cancels in the sumexp normalization.
  - k-bias dropped (constant per query -> cancels in softmax); v-bias folded
    into the conv2 shift on the host (softmax rows sum to 1).
  - sumexp reciprocal reads PSUM directly (no ACT-side row copy).
  - x, conv1 weights, CBAM spatial chain in bf16; final fuse on DVE with
    bf16 2x modes; conv2 z-add + channel sums on ACT via accum_out.
"""
import numpy as np
import ml_dtypes

import concourse.bacc as bacc
import concourse.bass as bass
import concourse.tile as tile
from concourse import mybir, bass_isa
from concourse.bass_utils import run_bass_kernel_spmd

# ---- custom DVE op: p = ((x + c0)^2 + c1)^32 -------------------------------
from concourse.dve_spec import Spec, Src0, C0, C1, Bin, AluOp, lower as _dve_lower
from concourse.dve_uop import DveOpSpec
import concourse.dve_ops as _dve_ops_mod
from concourse.dve_ops import DveOp as _DveOp


def _ref_exp32q(in0, in1, c0, c1, c2):
    f32 = np.float32
    t = in0.astype(f32)
    c0a = np.asarray(c0, f32)
    c1a = np.asarray(c1, f32)
    m = (t + c0a).astype(f32)
    p = (m * m).astype(f32)
    p = (p + c1a).astype(f32)
    for _ in range(5):
        p = (p * p).astype(f32)
    return p


def _register_exp32q():
    if "EXP32Q_ANT" in _dve_ops_mod.CUSTOM_DVE_SPECS:
        for op in _dve_ops_mod.OPS:
            if op.name == "EXP32Q_ANT":
                return op
    m = Src0 + C0
    p = Bin(AluOp.MULTIPLY, m, m) + C1
    for _ in range(5):
        p = Bin(AluOp.MULTIPLY, p, p)
    spec = Spec(body=p, reference=_ref_exp32q)
    row = _dve_ops_mod._CUSTOM_DVE_ROW_BASE + len(_dve_ops_mod.OPS)
    shas = {}
    for ver in ("v3", "v4"):
        uops = _dve_lower(spec, ver=ver)
        shas[ver] = DveOpSpec(name="EXP32Q_ANT", opcode=row, uops=uops,
                              rd1_en=False).sha(ver)
    op = _DveOp("EXP32Q_ANT", spec, subdim=False, uops_sha=shas)
    _dve_ops_mod.OPS.append(op)
    _dve_ops_mod.CUSTOM_DVE_SPECS[op.name] = op.spec
    _dve_ops_mod._SUB_OPCODE_FOR_NAME[op.name] = row
    return op


EXP32Q_ANT = _register_exp32q()

F32 = mybir.dt.float32
F32R = mybir.dt.float32r
BF16 = mybir.dt.bfloat16
FP8 = mybir.dt.float8e4

B, C, H, W = 16, 256, 32, 32
N = H * W          # 1024
NCORES = 8
SPC = B // NCORES  # samples per core = 2
NH, D = 4, 64      # heads, head dim
HS = 80            # padded head slot in v^T (16B-aligned stationary stride)
R = C // 8         # 32, channel attention bottleneck
EPS = 1e-5
DR = mybir.MatmulPerfMode.DoubleRow

# exp scaling: scores arrive as x = A2*(q.k) with A2 = sqc*ga/256;
# DVE computes ((x+C0)^2+C1)^32 = e^(qk/8 - 1.2); ACT uses
# exp(SCALE_ACT*x - 1.2).  The e^-1.2 shift cancels in normalization.
SHIFT = 1.2
SQC = float(np.exp(-SHIFT / 64.0))        # c^(1/2) with c = e^(-SHIFT/32)
GA = float(np.sqrt(2.0) / 2.0)
EXP_C0 = float(np.float32(SQC * GA))
EXP_C1 = float(np.float32(0.5 * np.exp(-SHIFT / 32.0)))
A2 = SQC * GA / 256.0                     # per-pair scale on q.k
ALPHA = float(np.sqrt(A2))                # fold into each of wq,bq,wk
SCALE_ACT = float(np.float32(32.0 / (SQC * GA)))

import os
# key blocks (mb) whose exp runs on the DVE custom op; rest on ACT
DVE_MBS = set(int(c) for c in os.environ.get("KNOB_DVE_MBS", "357"))


def build_module():
    nc = bacc.Bacc("TRN2", target_bir_lowering=False, debug=False)

    def din(name, shape, dt=F32):
        return nc.dram_tensor(name, shape, dt, kind="ExternalInput").ap()

    def dout(name, shape, dt=F32):
        return nc.dram_tensor(name, shape, dt, kind="ExternalOutput").ap()

    xr = din("xr", (SPC, 2, 128, N), BF16)        # per-sample x, c-blocks
    w1fT = din("w1fT", (2, 128, C), BF16)         # [kb][c,128 -> o cols]
    wqT = din("wqT", (2, 128, C), BF16)           # ALPHA-scaled
    wkT = din("wkT", (2, 128, C), BF16)
    wvT = din("wvT", (2, 128, C), BF16)           # gamma folded
    w2fT = din("w2fT", (2, 128, C), BF16)
    shift1 = din("shift1", (2, 128, 1), F32)
    bq_c = din("bq_c", (2, 128, 1), F32)          # per-plane bias (scaled)
    shift2 = din("shift2", (2, 128, 1), F32)      # bn2 shift + w2f@bvg
    caw1T = din("caw1T", (2, 128, 2 * R), F32)    # cols 0:32 avg(/1024), 32:64 max
    caw2T = din("caw2T", (2 * R, C), F32)
    wband = din("wband", (32, 14, 32), BF16)      # sa conv bands, (c2,kx)
    ones_in = din("ones_in", (128, 1), BF16)

    out = dout("out", (SPC, 2, 128, N), F32)
    scr_sa = dout("scr_sa", (SPC, 32, 32), BF16)  # bounce: sigmoid(sa)

    with tile.TileContext(nc) as tc:
        with (
            tc.tile_pool(name="wpool", bufs=1) as wp,
            tc.tile_pool(name="xpool", bufs=1) as xp,
            tc.tile_pool(name="rpool", bufs=1) as rp,
            tc.tile_pool(name="qkpool", bufs=1) as qkp,
            tc.tile_pool(name="vpool", bufs=1) as vp,
            tc.tile_pool(name="epool", bufs=13) as ep,
            tc.tile_pool(name="ypool", bufs=1) as yp,
            tc.tile_pool(name="zpool", bufs=2) as zp,
            tc.tile_pool(name="spool", bufs=2) as sp,
            tc.tile_pool(name="opool", bufs=2) as op_,
            tc.tile_pool(name="ps_sc", bufs=2, space="PSUM") as ps_sc,
            tc.tile_pool(name="ps_at", bufs=1, space="PSUM") as ps_at,
            tc.tile_pool(name="ps_a", bufs=2, space="PSUM") as ps_a,
        ):
            # ---- load order: conv1 weights + inputs first ----
            w1t = wp.tile([128, 2, C], BF16, tag="w1t", name="w1t")
            nc.sync.dma_start(out=w1t, in_=w1fT.rearrange("k p c -> p k c"))
            sh1 = wp.tile([128, 2], F32, tag="sh1", name="sh1")
            nc.sync.dma_start(out=sh1, in_=shift1.rearrange("k p a -> p (k a)"))
            xt_all = [[xp.tile([128, N], BF16, tag=f"x{si}{cb}", name=f"x{si}{cb}")
                       for cb in range(2)] for si in range(SPC)]
            for nch in range(2):
                for cb in range(2):
                    nc.sync.dma_start(
                        out=xt_all[0][cb][:, nch * 512:(nch + 1) * 512],
                        in_=xr[0, cb][:, nch * 512:(nch + 1) * 512])
            wqt = wp.tile([128, 2, C], BF16, tag="wqt", name="wqt")
            nc.sync.dma_start(out=wqt, in_=wqT.rearrange("k p c -> p k c"))
            wkt = wp.tile([128, 2, C], BF16, tag="wkt", name="wkt")
            nc.sync.dma_start(out=wkt, in_=wkT.rearrange("k p c -> p k c"))
            for cb in range(2):
                nc.sync.dma_start(out=xt_all[1][cb], in_=xr[1, cb])
            bqc = wp.tile([128, 2], F32, tag="bqc", name="bqc")
            nc.sync.dma_start(out=bqc, in_=bq_c.rearrange("k p a -> p (k a)"))
            wvt = wp.tile([128, 2, C], BF16, tag="wvt", name="wvt")
            nc.sync.dma_start(out=wvt, in_=wvT.rearrange("k p c -> p k c"))
            w2t = wp.tile([128, 2, C], BF16, tag="w2t", name="w2t")
            nc.sync.dma_start(out=w2t, in_=w2fT.rearrange("k p c -> p k c"))
            sh2 = wp.tile([128, 2], F32, tag="sh2", name="sh2")
            nc.sync.dma_start(out=sh2, in_=shift2.rearrange("k p a -> p (k a)"))
            cw1 = wp.tile([128, 2, 2 * R], F32, tag="cw1", name="cw1")
            nc.sync.dma_start(out=cw1, in_=caw1T.rearrange("k p c -> p k c"))
            cw2 = wp.tile([2 * R, C], F32, tag="cw2", name="cw2")
            nc.sync.dma_start(out=cw2, in_=caw2T)
            wbd = wp.tile([32, 14, 32], BF16, tag="wbd", name="wbd")
            nc.sync.dma_start(out=wbd, in_=wband)
            ones_fr = wp.tile([128, 1], BF16, tag="ones_fr", name="ones_fr")
            nc.sync.dma_start(out=ones_fr, in_=ones_in)
            nbias = wp.tile([128, 1], F32, tag="nbias", name="nbias")
            nc.vector.memset(nbias, -SHIFT)

            xt = [None] * SPC      # [s][cb] bf16 input tiles
            rt = [None] * SPC      # relu(conv1) tiles (bf16)
            qt = [None] * SPC      # [128, N] fp8 per ob
            kt = [None] * SPC
            vt = [None] * SPC      # vT fp8 [128, mb, NH*HS]
            ytmp = [None] * SPC    # y = attn + r (bf16)

            def a_conv(s):
                xt[s] = xt_all[s]
                rt[s] = [rp.tile([128, N], BF16, tag=f"r{s}{ob}", name=f"r{s}{ob}")
                         for ob in range(2)]
                for ob in range(2):
                    for nch in range(2):
                        pa = ps_a.tile([128, 512], F32, tag="pa", name="pa")
                        for kb in range(2):
                            nc.tensor.matmul(
                                pa, w1t[:, kb, ob * 128:(ob + 1) * 128],
                                xt[s][kb][:, nch * 512:(nch + 1) * 512],
                                start=(kb == 0), stop=(kb == 1))
                        nc.scalar.activation(
                            out=rt[s][ob][:, nch * 512:(nch + 1) * 512],
                            in_=pa, bias=sh1[:, ob:ob + 1], scale=1.0,
                            func=mybir.ActivationFunctionType.Relu)

            def a_qk(s, dst, wt, bc, on_act):
                for ob in range(2):
                    for nch in range(2):
                        pa = ps_a.tile([128, 512], F32, tag="pa", name="pa")
                        for kb in range(2):
                            nc.tensor.matmul(
                                pa, wt[:, kb, ob * 128:(ob + 1) * 128],
                                rt[s][kb][:, nch * 512:(nch + 1) * 512],
                                start=(kb == 0), stop=(kb == 1))
                        if on_act:
                            nc.scalar.activation(
                                out=dst[ob][:, nch * 512:(nch + 1) * 512],
                                in_=pa,
                                bias=(bc[:, ob:ob + 1] if bc is not None
                                      else 0.0),
                                scale=1.0,
                                func=(mybir.ActivationFunctionType.Identity
                                      if bc is not None else
                                      mybir.ActivationFunctionType.Copy))
                        else:
                            nc.vector.tensor_scalar(
                                dst[ob][:, nch * 512:(nch + 1) * 512], pa,
                                0.0, None, mybir.AluOpType.add)

            def a_q(s):
                qt[s] = [qkp.tile([128, N], FP8, tag=f"q{s}{ob}",
                                  name=f"q{s}{ob}") for ob in range(2)]
                a_qk(s, qt[s], wqt, bqc, True)

            def a_k(s):
                kt[s] = [qkp.tile([128, N], FP8, tag=f"k{s}{ob}",
                                  name=f"k{s}{ob}") for ob in range(2)]
                a_qk(s, kt[s], wkt, None, False)

            def a_vt(s):
                vt[s] = vp.tile([128, 8, NH * HS], FP8, tag=f"v{s}", name=f"v{s}")
                vv = vt[s].rearrange("p m (h e) -> p m h e", h=NH)
                nc.gpsimd.memset(vv[:, :, :, D:D + 1], 1.0)
                nc.gpsimd.memset(vv[:, :, :, D + 1:HS], 0.0)
                for mb in range(8):
                    pa = ps_a.tile([128, 512], F32, tag="pa", name="pa")
                    for kb in range(2):
                        nc.tensor.matmul(
                            pa[:, 0:C],
                            rt[s][kb][:, mb * 128:(mb + 1) * 128],
                            wvt[:, kb, :], start=(kb == 0), stop=(kb == 1))
                    nc.vector.tensor_scalar(
                        vv[:, mb, :, 0:D],
                        pa[:, 0:C].rearrange("p (h d) -> p h d", h=NH),
                        0.0, None, mybir.AluOpType.add)

            def phase_a(s):
                a_conv(s)
                a_q(s)
                a_k(s)
                a_vt(s)

            def phase_b(s, extras=()):
                ytmp[s] = [yp.tile([128, N], BF16, tag=f"yt{s}{pb}",
                                   name=f"yt{s}{pb}")
                           for pb in range(2)]
                et_all = [[None] * 4 for _ in range(NH)]  # per (h, mb-pair)

                def emit_se(h, mb):
                    # scores^T for key block mb: out [128 keys, 1024 queries]
                    pb, ro = h // 2, (h % 2) * 64
                    dsl = slice(ro, ro + 64)
                    psc = ps_sc.tile([128, 1024], F32, tag="psc", name="psc")
                    for nch in range(2):
                        nc.tensor.matmul(
                            psc[:, nch * 512:(nch + 1) * 512],
                            kt[s][pb][dsl, mb * 128:(mb + 1) * 128],
                            qt[s][pb][dsl, nch * 512:(nch + 1) * 512],
                            start=True, stop=True)
                    pr, half = mb // 2, mb % 2
                    if half == 0:
                        et_all[h][pr] = ep.tile([128, 2048], FP8, tag="et",
                                                name="et")
                    dst = et_all[h][pr][:, half * 1024:(half + 1) * 1024]
                    if mb in DVE_MBS:
                        nc.vector._custom_dve(EXP32Q_ANT, out=dst, in0=psc,
                                              s0=EXP_C0, s1=EXP_C1)
                    else:
                        nc.scalar.activation(
                            out=dst, in_=psc,
                            func=mybir.ActivationFunctionType.Exp,
                            scale=SCALE_ACT, bias=nbias)

                def emit_pv(h):
                    pb, ro = h // 2, (h % 2) * 64
                    pat = ps_at.tile([HS, N], F32, tag="pat", name="pat")
                    for nch in range(2):
                        for pr in range(4):
                            nc.tensor.matmul(
                                pat[:, nch * 512:(nch + 1) * 512],
                                vt[s][:, 2 * pr:2 * pr + 2,
                                      h * HS:(h + 1) * HS],
                                et_all[h][pr].rearrange(
                                    "p (t n) -> p t n", t=2)[
                                    :, :, nch * 512:(nch + 1) * 512],
                                start=(pr == 0), stop=(pr == 3),
                                perf_mode=DR)
                    # normalize: recip of sumexp row (read straight from
                    # PSUM), broadcast, multiply
                    srow = sp.tile([1, N], F32, tag="srow", name="srow", bufs=1)
                    rr = sp.tile([1, N], F32, tag="rr", name="rr", bufs=1)
                    rb = sp.tile([64, N], F32, tag="rb", name="rb", bufs=1)
                    nc.scalar.copy(srow, pat[D:D + 1, :])
                    nc.vector.reciprocal_approx_fast(out=rr, in_=srow)
                    nc.gpsimd.partition_broadcast(rb, rr, channels=64)
                    nc.vector.tensor_tensor(
                        out=ytmp[s][pb][ro:ro + 64, :],
                        in0=pat[0:D, :], in1=rb, op=mybir.AluOpType.mult)

                PF = 6
                for h in range(NH):
                    for mb in (range(PF, 8) if h > 0 else range(8)):
                        emit_se(h, mb)
                    if h + 1 < NH:
                        for mb in range(PF):
                            emit_se(h + 1, mb)
                    emit_pv(h)
                    if h % 2 == 1:
                        # y = attn + r for finished pb (both bf16 -> DVE 2x)
                        pb = h // 2
                        nc.vector.tensor_tensor(
                            out=ytmp[s][pb], in0=ytmp[s][pb], in1=rt[s][pb],
                            op=mybir.AluOpType.add)
                    if extras and h < len(extras):
                        extras[h]()

            def c_conv(s):
                # conv2 + bn2 -> z (fp32) on ACT, with per-channel sums for
                # CBAM avg via accum_out
                zt = [zp.tile([128, N], F32, tag=f"z{ob}", name=f"z{ob}")
                      for ob in range(2)]
                cols = [zp.tile([128, 2], F32, tag=f"cols{ob}", name=f"cols{ob}")
                        for ob in range(2)]
                accs = [zp.tile([128, 2], F32, tag=f"acc{ob}", name=f"acc{ob}")
                        for ob in range(2)]
                for ob in range(2):
                    for nch in range(2):
                        pa = ps_a.tile([128, 512], F32, tag="pa", name="pa")
                        for kb in range(2):
                            nc.tensor.matmul(
                                pa, w2t[:, kb, ob * 128:(ob + 1) * 128],
                                ytmp[s][kb][:, nch * 512:(nch + 1) * 512],
                                start=(kb == 0), stop=(kb == 1))
                        if s == 1:
                            nc.scalar.activation(
                                out=zt[ob][:, nch * 512:(nch + 1) * 512],
                                in_=pa, bias=sh2[:, ob:ob + 1], scale=1.0,
                                func=mybir.ActivationFunctionType.Identity,
                                accum_out=accs[ob][:, nch:nch + 1])
                        else:
                            nc.vector.tensor_scalar(
                                zt[ob][:, nch * 512:(nch + 1) * 512], pa,
                                sh2[:, ob:ob + 1], 0.0,
                                mybir.AluOpType.add, mybir.AluOpType.add,
                                accum_out=accs[ob][:, nch:nch + 1])
                    # combine the two 512-chunk sums into col 0
                    nc.vector.tensor_tensor(
                        out=cols[ob][:, 0:1], in0=accs[ob][:, 0:1],
                        in1=accs[ob][:, 1:2], op=mybir.AluOpType.add)
                    # channel max into col 1
                    nc.vector.tensor_reduce(
                        out=cols[ob][:, 1:2], in_=zt[ob],
                        op=mybir.AluOpType.max,
                        axis=mybir.AxisListType.X)
                return zt, cols

            def c_chan(s, zt, cols):
                # channel attention: one fused FC1 matmul pair gives
                # [W1a@sum | W1m@max] stacked on 64 partitions; FC2 uses a
                # host-duplicated cw2 so both halves contract in place
                ph = ps_a.tile([128, 512], F32, tag="pa", name="pa")
                for kb in range(2):
                    nc.tensor.matmul(ph[0:2 * R, 0:2], cw1[:, kb, :],
                                     cols[kb][:, 0:2],
                                     start=(kb == 0), stop=(kb == 1))
                hsb = sp.tile([2 * R, 2], F32, tag="hsb", name="hsb")
                nc.vector.tensor_scalar(hsb, ph[0:2 * R, 0:2], 0.0, None,
                                        mybir.AluOpType.max)
                ca = [sp.tile([128, 1], F32, tag=f"ca{ob}", name=f"ca{ob}")
                      for ob in range(2)]
                for ob in range(2):
                    pc = ps_a.tile([128, 512], F32, tag="pa", name="pa")
                    nc.tensor.matmul(pc[:, 0:1],
                                     cw2[0:R, ob * 128:(ob + 1) * 128],
                                     hsb[0:R, 0:1], start=True, stop=False)
                    nc.tensor.matmul(pc[:, 0:1],
                                     cw2[R:2 * R, ob * 128:(ob + 1) * 128],
                                     hsb[R:2 * R, 1:2], start=False, stop=True)
                    nc.scalar.activation(
                        out=ca[ob], in_=pc[:, 0:1],
                        func=mybir.ActivationFunctionType.Tanh, scale=0.5)
                    nc.vector.tensor_scalar(
                        ca[ob], ca[ob], 0.5, 0.5,
                        mybir.AluOpType.mult, mybir.AluOpType.add)
                # apply channel attention -> z_ca (bf16)
                zca = [zp.tile([128, N], BF16, tag=f"zca{ob}", name=f"zca{ob}")
                       for ob in range(2)]
                for ob in range(2):
                    if s == 1:
                        nc.scalar.activation(
                            out=zca[ob], in_=zt[ob],
                            func=mybir.ActivationFunctionType.Copy,
                            scale=ca[ob])
                    else:
                        nc.vector.tensor_scalar_mul(zca[ob], zt[ob], ca[ob])
                return zca

            def c_spat(s, zca):
                # spatial sum (avg path) via ones matmul (bf16)
                avg_row = sp.tile([1, N], BF16, tag="avg_row", name="avg_row",
                                  bufs=1)
                for nch in range(2):
                    psr = ps_a.tile([128, 512], F32, tag="pa", name="pa")
                    for kb in range(2):
                        nc.tensor.matmul(
                            psr[0:1, :],
                            ones_fr, zca[kb][:, nch * 512:(nch + 1) * 512],
                            start=(kb == 0), stop=(kb == 1))
                    nc.scalar.copy(
                        avg_row[:, nch * 512:(nch + 1) * 512], psr[0:1, :])
                # spatial max via TT max (bf16 2x) + gpsimd reduce
                m1 = zp.tile([128, N], BF16, tag="m1", name="m1", bufs=1)
                nc.vector.tensor_tensor(out=m1, in0=zca[0], in1=zca[1],
                                        op=mybir.AluOpType.max)
                rep = zp.tile([128, N], BF16, tag="rep", name="rep", bufs=1)
                nc.gpsimd.partition_all_reduce(rep, m1, channels=128,
                                               reduce_op=bass_isa.ReduceOp.max)
                # reshape rows [1, 1024] -> [32(y), 32(x)] via SBUF->SBUF DMA
                avgT = sp.tile([32, 38], BF16, tag="avgT", name="avgT")
                nc.gpsimd.memset(avgT, 0.0)
                nc.sync.dma_start(out=avgT[:, 3:35], in_=avg_row)
                maxT = sp.tile([32, 38], BF16, tag="maxT", name="maxT")
                nc.gpsimd.memset(maxT, 0.0)
                nc.sync.dma_start(out=maxT[:, 3:35], in_=rep[0:1, :])
                # 7x7 conv as 14 banded matmuls over y, x-shifts on free dim
                psa = ps_a.tile([128, 512], F32, tag="pa", name="pa")
                first = True
                for c2, inp in ((0, avgT), (1, maxT)):
                    for kx in range(7):
                        nc.tensor.matmul(
                            psa[0:32, 0:32],
                            wbd[:, c2 * 7 + kx, :],
                            inp[:, kx:kx + 32],
                            start=first, stop=(c2 == 1 and kx == 6))
                        first = False
                sasb = sp.tile([32, 32], BF16, tag="sasb", name="sasb")
                nc.scalar.activation(
                    out=sasb, in_=psa[0:32, 0:32],
                    func=mybir.ActivationFunctionType.Tanh, scale=0.5)
                nc.vector.tensor_scalar(
                    sasb, sasb, 0.5, 0.5,
                    mybir.AluOpType.mult, mybir.AluOpType.add)
                sarow = sp.tile([1, N], BF16, tag="sarow", name="sarow")
                nc.sync.dma_start(out=sarow, in_=sasb)
                sarep = zp.tile([128, N], BF16, tag="sarep", name="sarep",
                                bufs=1)
                nc.gpsimd.partition_broadcast(sarep, sarow, channels=128)
                return sarep

            def c_fin(s, zca, sarep):
                # final: out = relu(z_ca * sa + x); cb0 TTs on DVE, cb1 TTs
                # on Pool so both channel blocks run in parallel in the tail
                us = [op_.tile([128, N], BF16, tag=f"fin{cb}", name=f"fin{cb}")
                      for cb in range(2)]
                os_ = [op_.tile([128, N], F32, tag=f"fino{cb}",
                                name=f"fino{cb}") for cb in range(2)]
                for ch in range(2):
                    lo, hi = ch * 512, ch * 512 + 512
                    for cb in range(2):
                        eng = nc.gpsimd if (cb == 1 and ch == 1) else nc.vector
                        u, o = us[cb], os_[cb]
                        eng.tensor_tensor(
                            out=u[:, lo:hi], in0=zca[cb][:, lo:hi],
                            in1=sarep[:, lo:hi], op=mybir.AluOpType.mult)
                        eng.tensor_tensor(
                            out=u[:, lo:hi], in0=u[:, lo:hi],
                            in1=xt[s][cb][:, lo:hi], op=mybir.AluOpType.add)
                        nc.vector.tensor_scalar(
                            o[:, lo:hi], u[:, lo:hi], 0.0, None,
                            mybir.AluOpType.max)
                        nc.sync.dma_start(out=out[s, cb][:, lo:hi],
                                          in_=o[:, lo:hi])

            def phase_c(s):
                zt, cols = c_conv(s)
                zca = c_chan(s, zt, cols)
                sarep = c_spat(s, zca)
                c_fin(s, zca, sarep)

            phase_a(0)
            phase_b(0, extras=(lambda: a_conv(1), lambda: a_q(1),
                               lambda: a_k(1), lambda: a_vt(1)))
            c0_state = {}

            def x0():
                c0_state["zt"], c0_state["cols"] = c_conv(0)

            def x1():
                c0_state["zca"] = c_chan(0, c0_state["zt"], c0_state["cols"])

            def x2():
                c0_state["sarep"] = c_spat(0, c0_state["zca"])

            def x3():
                c_fin(0, c0_state["zca"], c0_state["sarep"])

            phase_b(1, extras=(x0, x1, x2, x3))
            phase_c(1)

    nc.compile()
    return nc


_NC_CACHE = None


def get_module():
    global _NC_CACHE
    if _NC_CACHE is None:
        _NC_CACHE = build_module()
    return _NC_CACHE


def prep_inputs(x, w1, bn1_g, bn1_b, bn1_m, bn1_v, wq, bq, wk, bk, wv, bv,
                gamma, w2, bn2_g, bn2_b, bn2_m, bn2_v, ca_w1, ca_w2, sa_w):
    """Host-side preprocessing -> per-core in_maps."""
    f64 = np.float64
    bf16 = ml_dtypes.bfloat16
    s1 = (bn1_g.astype(f64) / np.sqrt(bn1_v.astype(f64) + EPS))
    w1f = (s1[:, None] * w1.astype(f64)).astype(np.float32)
    sh1 = (bn1_b.astype(f64) - bn1_m.astype(f64) * s1).astype(np.float32)
    s2 = (bn2_g.astype(f64) / np.sqrt(bn2_v.astype(f64) + EPS))
    w2f = (s2[:, None] * w2.astype(f64)).astype(np.float32)
    g = float(gamma[0])
    wvg = (wv.astype(f64) * g).astype(np.float32)
    bvg = (bv.astype(f64) * g)
    # v-bias folds into the conv2 shift: softmax rows sum to 1, so the
    # attention output gets exactly +bvg per channel before conv2.
    sh2 = (bn2_b.astype(f64) - bn2_m.astype(f64) * s2
           + (s2[:, None] * w2.astype(f64)) @ bvg).astype(np.float32)

    def lhsT(w):  # [O, C] -> [2, 128, O] kb-blocked transpose, bf16
        return np.ascontiguousarray(w.T.reshape(2, 128, C)).astype(bf16)

    def lhsT_qk(w):
        wt = (w.astype(f64) * ALPHA).astype(np.float32).T  # [C_in, O]
        return np.ascontiguousarray(wt.reshape(2, 128, C)).astype(bf16)

    def bias_qk(b):
        bb = (b.astype(f64) * ALPHA).astype(np.float32)
        return np.ascontiguousarray(bb.reshape(2, 128, 1))

    base = {
        "w1fT": lhsT(w1f),
        "wqT": lhsT_qk(wq),
        "wkT": lhsT_qk(wk),
        "wvT": np.ascontiguousarray(wvg.T.reshape(2, 128, C)).astype(bf16),
        "w2fT": lhsT(w2f),
        "shift1": np.ascontiguousarray(sh1.reshape(2, 128, 1)),
        "bq_c": bias_qk(bq),
        "shift2": np.ascontiguousarray(sh2.reshape(2, 128, 1)),
    }
    # channel attention weights: caw1T [2, 128, 64]
    c1T = ca_w1.T.astype(np.float32)             # [C, R]
    caw1T = np.concatenate([c1T / float(N), c1T], axis=1)  # [C, 2R]
    base["caw1T"] = np.ascontiguousarray(caw1T.reshape(2, 128, 2 * R))
    c2T = ca_w2.T.astype(np.float32)             # [R, C]
    base["caw2T"] = np.ascontiguousarray(np.concatenate([c2T, c2T], axis=0))
    # spatial conv bands: wband[yi, c2*7+kx, yo] = w[c2, yi-yo+3, kx]
    wb = np.zeros((32, 14, 32), np.float32)
    for c2 in range(2):
        for kx in range(7):
            for yo in range(32):
                for ky in range(7):
                    yi = yo + ky - 3
                    if 0 <= yi < 32:
                        v = sa_w[0, c2, ky, kx]
                        if c2 == 0:
                            v = v / float(C)
                        wb[yi, c2 * 7 + kx, yo] = v
    base["wband"] = wb.astype(bf16)
    base["ones_in"] = np.ones((128, 1), bf16)

    xrf = x.reshape(B, C, N).astype(bf16)
    in_maps = []
    for core in range(NCORES):
        m = dict(base)
        m["xr"] = np.ascontiguousarray(
            xrf[core * SPC:(core + 1) * SPC].reshape(SPC, 2, 128, N))
        in_maps.append(m)
    return in_maps


def kernel(**inputs):
    nc = get_module()
    in_maps = prep_inputs(**inputs)
    res = run_bass_kernel_spmd(nc, in_maps, core_ids=list(range(NCORES)))
    outs = []
    for core in range(NCORES):
        o = res.results[core]["out"]  # [SPC, 2, 128, N]
        outs.append(o.reshape(SPC, C, H, W))
    return np.concatenate(outs, axis=0)


if __name__ == "__main__":
    nc = get_module()
    print("compiled ok")


# revision 21
# speedup vs baseline: 1.0416x; 1.0034x over previous
"""Trainium2 Bass kernel for nn_BottleneckTransformer.

Data-parallel over batch: B=16 samples -> 8 cores x 2 samples.
Key optimizations over the f32r/bf16 baseline:
  - scores^T and PV matmuls in fp8e4 with DoubleRow perf mode; q/k stored
    plane-packed, v^T stored with 80-wide head slots.
  - softmax exp split across ACT (table exp) and DVE (custom EXP32Q_ANT,
    p=(m^2+c)^32 ~ e^(s-1.2)); the e^-1.2 shift avoids fp8e4 overflow and
    cancels in the sumexp normalization.
  - k-bias dropped (constant per query -> cancels in softmax); v-bias folded
    into the conv2 shift on the host (softmax rows sum to 1).
  - sumexp reciprocal reads PSUM directly (no ACT-side row copy).
  - x, conv1 weights, CBAM spatial chain in bf16; final fuse on DVE with
    bf16 2x modes; conv2 z-add + channel sums on ACT via accum_out.
"""
import numpy as np
import ml_dtypes

import concourse.bacc as bacc
import concourse.bass as bass
import concourse.tile as tile
from concourse import mybir, bass_isa
from concourse.bass_utils import run_bass_kernel_spmd

# ---- custom DVE op: p = ((x + c0)^2 + c1)^32 -------------------------------
from concourse.dve_spec import Spec, Src0, C0, C1, Bin, AluOp, lower as _dve_lower
from concourse.dve_uop import DveOpSpec
import concourse.dve_ops as _dve_ops_mod
from concourse.dve_ops import DveOp as _DveOp


def _ref_exp32q(in0, in1, c0, c1, c2):
    f32 = np.float32
    t = in0.astype(f32)
    c0a = np.asarray(c0, f32)
    c1a = np.asarray(c1, f32)
    m = (t + c0a).astype(f32)
    p = (m * m).astype(f32)
    p = (p + c1a).astype(f32)
    for _ in range(5):
        p = (p * p).astype(f32)
    return p


def _register_exp32q():
    if "EXP32Q_ANT" in _dve_ops_mod.CUSTOM_DVE_SPECS:
        for op in _dve_ops_mod.OPS:
            if op.name == "EXP32Q_ANT":
                return op
    m = Src0 + C0
    p = Bin(AluOp.MULTIPLY, m, m) + C1
    for _ in range(5):
        p = Bin(AluOp.MULTIPLY, p, p)
    spec = Spec(body=p, reference=_ref_exp32q)
    row = _dve_ops_mod._CUSTOM_DVE_ROW_BASE + len(_dve_ops_mod.OPS)
    shas = {}
    for ver in ("v3", "v4"):
        uops = _dve_lower(spec, ver=ver)
        shas[ver] = DveOpSpec(name="EXP32Q_ANT", opcode=row, uops=uops,
                              rd1_en=False).sha(ver)
    op = _DveOp("EXP32Q_ANT", spec, subdim=False, uops_sha=shas)
    _dve_ops_mod.OPS.append(op)
    _dve_ops_mod.CUSTOM_DVE_SPECS[op.name] = op.spec
    _dve_ops_mod._SUB_OPCODE_FOR_NAME[op.name] = row
    return op


EXP32Q_ANT = _register_exp32q()

F32 = mybir.dt.float32
F32R = mybir.dt.float32r
BF16 = mybir.dt.bfloat16
FP8 = mybir.dt.float8e4

B, C, H, W = 16, 256, 32, 32
N = H * W          # 1024
NCORES = 8
SPC = B // NCORES  # samples per core = 2
NH, D = 4, 64      # heads, head dim
HS = 80            # padded head slot in v^T (16B-aligned stationary stride)
R = C // 8         # 32, channel attention bottleneck
EPS = 1e-5
DR = mybir.MatmulPerfMode.DoubleRow

# exp scaling: scores arrive as x = A2*(q.k) with A2 = sqc*ga/256;
# DVE computes ((x+C0)^2+C1)^32 = e^(qk/8 - 1.2); ACT uses
# exp(SCALE_ACT*x - 1.2).  The e^-1.2 shift cancels in normalization.
SHIFT = 1.2
SQC = float(np.exp(-SHIFT / 64.0))        # c^(1/2) with c = e^(-SHIFT/32)
GA = float(np.sqrt(2.0) / 2.0)
EXP_C0 = float(np.float32(SQC * GA))
EXP_C1 = float(np.float32(0.5 * np.exp(-SHIFT / 32.0)))
A2 = SQC * GA / 256.0                     # per-pair scale on q.k
ALPHA = float(np.sqrt(A2))                # fold into each of wq,bq,wk
SCALE_ACT = float(np.float32(32.0 / (SQC * GA)))

import os
# key blocks (mb) whose exp runs on the DVE custom op; rest on ACT
DVE_MBS = set(int(c) for c in os.environ.get("KNOB_DVE_MBS", "357"))


def build_module():
    nc = bacc.Bacc("TRN2", target_bir_lowering=False, debug=False)

    def din(name, shape, dt=F32):
        return nc.dram_tensor(name, shape, dt, kind="ExternalInput").ap()

    def dout(name, shape, dt=F32):
        return nc.dram_tensor(name, shape, dt, kind="ExternalOutput").ap()

    xr = din("xr", (SPC, 2, 128, N), BF16)        # per-sample x, c-blocks
    w1fT = din("w1fT", (2, 128, C), BF16)         # [kb][c,128 -> o cols]
    wqT = din("wqT", (2, 128, C), BF16)           # ALPHA-scaled
    wkT = din("wkT", (2, 128, C), BF16)
    wvT = din("wvT", (2, 128, C), BF16)           # gamma folded
    w2fT = din("w2fT", (2, 128, C), BF16)
    shift1 = din("shift1", (2, 128, 1), F32)
    bq_c = din("bq_c", (2, 128, 1), F32)          # per-plane bias (scaled)
    shift2 = din("shift2", (2, 128, 1), F32)      # bn2 shift + w2f@bvg
    caw1T = din("caw1T", (2, 128, 2 * R), F32)    # cols 0:32 avg(/1024), 32:64 max
    caw2T = din("caw2T", (2 * R, C), F32)
    wband = din("wband", (32, 14, 32), BF16)      # sa conv bands, (c2,kx)
    ones_in = din("ones_in", (128, 1), BF16)

    out = dout("out", (SPC, 2, 128, N), F32)
    scr_sa = dout("scr_sa", (SPC, 32, 32), BF16)  # bounce: sigmoid(sa)

    with tile.TileContext(nc) as tc:
        with (
            tc.tile_pool(name="wpool", bufs=1) as wp,
            tc.tile_pool(name="xpool", bufs=1) as xp,
            tc.tile_pool(name="rpool", bufs=1) as rp,
            tc.tile_pool(name="qkpool", bufs=1) as qkp,
            tc.tile_pool(name="vpool", bufs=1) as vp,
            tc.tile_pool(name="epool", bufs=13) as ep,
            tc.tile_pool(name="ypool", bufs=1) as yp,
            tc.tile_pool(name="zpool", bufs=2) as zp,
            tc.tile_pool(name="spool", bufs=2) as sp,
            tc.tile_pool(name="opool", bufs=2) as op_,
            tc.tile_pool(name="ps_sc", bufs=2, space="PSUM") as ps_sc,
            tc.tile_pool(name="ps_at", bufs=1, space="PSUM") as ps_at,
            tc.tile_pool(name="ps_a", bufs=2, space="PSUM") as ps_a,
        ):
            # ---- load order: conv1 weights + inputs first ----
            w1t = wp.tile([128, 2, C], BF16, tag="w1t", name="w1t")
            nc.sync.dma_start(out=w1t, in_=w1fT.rearrange("k p c -> p k c"))
            sh1 = wp.tile([128, 2], F32, tag="sh1", name="sh1")
            nc.sync.dma_start(out=sh1, in_=shift1.rearrange("k p a -> p (k a)"))
            xt_all = [[xp.tile([128, N], BF16, tag=f"x{si}{cb}", name=f"x{si}{cb}")
                       for cb in range(2)] for si in range(SPC)]
            for nch in range(2):
                for cb in range(2):
                    nc.sync.dma_start(
                        out=xt_all[0][cb][:, nch * 512:(nch + 1) * 512],
                        in_=xr[0, cb][:, nch * 512:(nch + 1) * 512])
            wqt = wp.tile([128, 2, C], BF16, tag="wqt", name="wqt")
            nc.sync.dma_start(out=wqt, in_=wqT.rearrange("k p c -> p k c"))
            wkt = wp.tile([128, 2, C], BF16, tag="wkt", name="wkt")
            nc.sync.dma_start(out=wkt, in_=wkT.rearrange("k p c -> p k c"))
            for cb in range(2):
                nc.sync.dma_start(out=xt_all[1][cb], in_=xr[1, cb])
            bqc = wp.tile([128, 2], F32, tag="bqc", name="bqc")
            nc.sync.dma_start(out=bqc, in_=bq_c.rearrange("k p a -> p (k a)"))
            wvt = wp.tile([128, 2, C], BF16, tag="wvt", name="wvt")
            nc.sync.dma_start(out=wvt, in_=wvT.rearrange("k p c -> p k c"))
            w2t = wp.tile([128, 2, C], BF16, tag="w2t", name="w2t")
            nc.sync.dma_start(out=w2t, in_=w2fT.rearrange("k p c -> p k c"))
            sh2 = wp.tile([128, 2], F32, tag="sh2", name="sh2")
            nc.sync.dma_start(out=sh2, in_=shift2.rearrange("k p a -> p (k a)"))
            cw1 = wp.tile([128, 2, 2 * R], F32, tag="cw1", name="cw1")
            nc.sync.dma_start(out=cw1, in_=caw1T.rearrange("k p c -> p k c"))
            cw2 = wp.tile([2 * R, C], F32, tag="cw2", name="cw2")
            nc.sync.dma_start(out=cw2, in_=caw2T)
            wbd = wp.tile([32, 14, 32], BF16, tag="wbd", name="wbd")
            nc.sync.dma_start(out=wbd, in_=wband)
            ones_fr = wp.tile([128, 1], BF16, tag="ones_fr", name="ones_fr")
            nc.sync.dma_start(out=ones_fr, in_=ones_in)
            nbias = wp.tile([128, 1], F32, tag="nbias", name="nbias")
            nc.vector.memset(nbias, -SHIFT)

            xt = [None] * SPC      # [s][cb] bf16 input tiles
            rt = [None] * SPC      # relu(conv1) tiles (bf16)
            qt = [None] * SPC      # [128, N] fp8 per ob
            kt = [None] * SPC
            vt = [None] * SPC      # vT fp8 [128, mb, NH*HS]
            ytmp = [None] * SPC    # y = attn + r (bf16)

            def a_conv(s):
                xt[s] = xt_all[s]
                rt[s] = [rp.tile([128, N], BF16, tag=f"r{s}{ob}", name=f"r{s}{ob}")
                         for ob in range(2)]
                for ob in range(2):
                    for nch in range(2):
                        pa = ps_a.tile([128, 512], F32, tag="pa", name="pa")
                        for kb in range(2):
                            nc.tensor.matmul(
                                pa, w1t[:, kb, ob * 128:(ob + 1) * 128],
                                xt[s][kb][:, nch * 512:(nch + 1) * 512],
                                start=(kb == 0), stop=(kb == 1))
                        nc.scalar.activation(
                            out=rt[s][ob][:, nch * 512:(nch + 1) * 512],
                            in_=pa, bias=sh1[:, ob:ob + 1], scale=1.0,
                            func=mybir.ActivationFunctionType.Relu)

            def a_qk(s, dst, wt, bc, on_act):
                for ob in range(2):
                    for nch in range(2):
                        pa = ps_a.tile([128, 512], F32, tag="pa", name="pa")
                        for kb in range(2):
                            nc.tensor.matmul(
                                pa, wt[:, kb, ob * 128:(ob + 1) * 128],
                                rt[s][kb][:, nch * 512:(nch + 1) * 512],
                                start=(kb == 0), stop=(kb == 1))
                        if on_act:
                            nc.scalar.activation(
                                out=dst[ob][:, nch * 512:(nch + 1) * 512],
                                in_=pa,
                                bias=(bc[:, ob:ob + 1] if bc is not None
                                      else 0.0),
                                scale=1.0,
                                func=(mybir.ActivationFunctionType.Identity
                                      if bc is not None else
                                      mybir.ActivationFunctionType.Copy))
                        else:
                            nc.vector.tensor_scalar(
                                dst[ob][:, nch * 512:(nch + 1) * 512], pa,
                                0.0, None, mybir.AluOpType.add)

            def a_q(s):
                qt[s] = [qkp.tile([128, N], FP8, tag=f"q{s}{ob}",
                                  name=f"q{s}{ob}") for ob in range(2)]
                a_qk(s, qt[s], wqt, bqc, True)

            def a_k(s):
                kt[s] = [qkp.tile([128, N], FP8, tag=f"k{s}{ob}",
                                  name=f"k{s}{ob}") for ob in range(2)]
                a_qk(s, kt[s], wkt, None, False)

            def a_vt(s):
                vt[s] = vp.tile([128, 8, NH * HS], FP8, tag=f"v{s}", name=f"v{s}")
                vv = vt[s].rearrange("p m (h e) -> p m h e", h=NH)
                nc.gpsimd.memset(vv[:, :, :, D:D + 1], 1.0)
                nc.gpsimd.memset(vv[:, :, :, D + 1:HS], 0.0)
                for mb in range(8):
                    pa = ps_a.tile([128, 512], F32, tag="pa", name="pa")
                    for kb in range(2):
                        nc.tensor.matmul(
                            pa[:, 0:C],
                            rt[s][kb][:, mb * 128:(mb + 1) * 128],
                            wvt[:, kb, :], start=(kb == 0), stop=(kb == 1))
                    nc.vector.tensor_scalar(
                        vv[:, mb, :, 0:D],
                        pa[:, 0:C].rearrange("p (h d) -> p h d", h=NH),
                        0.0, None, mybir.AluOpType.add)

            def phase_a(s):
                a_conv(s)
                a_q(s)
                a_k(s)
                a_vt(s)

            def phase_b(s, extras=()):
                ytmp[s] = [yp.tile([128, N], BF16, tag=f"yt{s}{pb}",
                                   name=f"yt{s}{pb}")
                           for pb in range(2)]
                et_all = [[None] * 4 for _ in range(NH)]  # per (h, mb-pair)

                def emit_se(h, mb):
                    # scores^T for key block mb: out [128 keys, 1024 queries]
                    pb, ro = h // 2, (h % 2) * 64
                    dsl = slice(ro, ro + 64)
                    psc = ps_sc.tile([128, 1024], F32, tag="psc", name="psc")
                    for nch in range(2):
                        nc.tensor.matmul(
                            psc[:, nch * 512:(nch + 1) * 512],
                            kt[s][pb][dsl, mb * 128:(mb + 1) * 128],
                            qt[s][pb][dsl, nch * 512:(nch + 1) * 512],
                            start=True, stop=True)
                    pr, half = mb // 2, mb % 2
                    if half == 0:
                        et_all[h][pr] = ep.tile([128, 2048], FP8, tag="et",
                                                name="et")
                    dst = et_all[h][pr][:, half * 1024:(half + 1) * 1024]
                    if mb in DVE_MBS:
                        nc.vector._custom_dve(EXP32Q_ANT, out=dst, in0=psc,
                                              s0=EXP_C0, s1=EXP_C1)
                    else:
                        nc.scalar.activation(
                            out=dst, in_=psc,
                            func=mybir.ActivationFunctionType.Exp,
                            scale=SCALE_ACT, bias=nbias)

                def emit_pv(h):
                    pb, ro = h // 2, (h % 2) * 64
                    pat = ps_at.tile([HS, N], F32, tag="pat", name="pat")
                    for nch in range(2):
                        for pr in range(4):
                            nc.tensor.matmul(
                                pat[:, nch * 512:(nch + 1) * 512],
                                vt[s][:, 2 * pr:2 * pr + 2,
                                      h * HS:(h + 1) * HS],
                                et_all[h][pr].rearrange(
                                    "p (t n) -> p t n", t=2)[
                                    :, :, nch * 512:(nch + 1) * 512],
                                start=(pr == 0), stop=(pr == 3),
                                perf_mode=DR)
                    # normalize: recip of sumexp row (read straight from
                    # PSUM), broadcast, multiply
                    srow = sp.tile([1, N], F32, tag="srow", name="srow", bufs=1)
                    rr = sp.tile([1, N], F32, tag="rr", name="rr", bufs=1)
                    rb = sp.tile([64, N], F32, tag="rb", name="rb", bufs=1)
                    nc.scalar.copy(srow, pat[D:D + 1, :])
                    nc.vector.reciprocal_approx_fast(out=rr, in_=srow)
                    nc.gpsimd.partition_broadcast(rb, rr, channels=64)
                    nc.vector.tensor_tensor(
                        out=ytmp[s][pb][ro:ro + 64, :],
                        in0=pat[0:D, :], in1=rb, op=mybir.AluOpType.mult)

                PF = 6
                for h in range(NH):
                    for mb in (range(PF, 8) if h > 0 else range(8)):
                        emit_se(h, mb)
                    if h + 1 < NH:
                        for mb in range(PF):
                            emit_se(h + 1, mb)
                    emit_pv(h)
                    if h % 2 == 1:
                        # y = attn + r for finished pb (both bf16 -> DVE 2x)
                        pb = h // 2
                        nc.vector.tensor_tensor(
                            out=ytmp[s][pb], in0=ytmp[s][pb], in1=rt[s][pb],
                            op=mybir.AluOpType.add)
                    if extras and h < len(extras):
                        extras[h]()

            def c_conv(s):
                # conv2 + bn2 -> z (fp32) on ACT, with per-channel sums for
                # CBAM avg via accum_out
                zt = [zp.tile([128, N], F32, tag=f"z{ob}", name=f"z{ob}")
                      for ob in range(2)]
                cols = [zp.tile([128, 2], F32, tag=f"cols{ob}", name=f"cols{ob}")
                        for ob in range(2)]
                accs = [zp.tile([128, 2], F32, tag=f"acc{ob}", name=f"acc{ob}")
                        for ob in range(2)]
                for ob in range(2):
                    for nch in range(2):
                        pa = ps_a.tile([128, 512], F32, tag="pa", name="pa")
                        for kb in range(2):
                            nc.tensor.matmul(
                                pa, w2t[:, kb, ob * 128:(ob + 1) * 128],
                                ytmp[s][kb][:, nch * 512:(nch + 1) * 512],
                                start=(kb == 0), stop=(kb == 1))
                        if s == 1:
                            nc.scalar.activation(
                                out=zt[ob][:, nch * 512:(nch + 1) * 512],
                                in_=pa, bias=sh2[:, ob:ob + 1], scale=1.0,
                                func=mybir.ActivationFunctionType.Identity,
                                accum_out=accs[ob][:, nch:nch + 1])
                        else:
                            nc.vector.tensor_scalar(
                                zt[ob][:, nch * 512:(nch + 1) * 512], pa,
                                sh2[:, ob:ob + 1], 0.0,
                                mybir.AluOpType.add, mybir.AluOpType.add,
                                accum_out=accs[ob][:, nch:nch + 1])
                    # combine the two 512-chunk sums into col 0
                    nc.vector.tensor_tensor(
                        out=cols[ob][:, 0:1], in0=accs[ob][:, 0:1],
                        in1=accs[ob][:, 1:2], op=mybir.AluOpType.add)
                    # channel max into col 1
                    nc.vector.tensor_reduce(
                        out=cols[ob][:, 1:2], in_=zt[ob],
                        op=mybir.AluOpType.max,
                        axis=mybir.AxisListType.X)
                return zt, cols

            def c_chan(s, zt, cols):
                # channel attention: h = relu(W1a@sum | W1m@max), ca=sig(W2@h)
                ph_a = ps_a.tile([128, 512], F32, tag="pa", name="pa")
                for kb in range(2):
                    nc.tensor.matmul(ph_a[0:R, 0:1], cw1[:, kb, 0:R],
                                     cols[kb][:, 0:1],
                                     start=(kb == 0), stop=(kb == 1))
                ph_m = ps_a.tile([128, 512], F32, tag="pa", name="pa")
                for kb in range(2):
                    nc.tensor.matmul(ph_m[0:R, 0:1], cw1[:, kb, R:2 * R],
                                     cols[kb][:, 1:2],
                                     start=(kb == 0), stop=(kb == 1))
                hsb = sp.tile([R, 2], F32, tag="hsb", name="hsb")
                nc.vector.tensor_scalar(hsb[:, 0:1], ph_a[0:R, 0:1], 0.0, None,
                                        mybir.AluOpType.max)
                nc.vector.tensor_scalar(hsb[:, 1:2], ph_m[0:R, 0:1], 0.0, None,
                                        mybir.AluOpType.max)
                ca = [sp.tile([128, 1], F32, tag=f"ca{ob}", name=f"ca{ob}")
                      for ob in range(2)]
                for ob in range(2):
                    pc = ps_a.tile([128, 512], F32, tag="pa", name="pa")
                    nc.tensor.matmul(pc[:, 0:1],
                                     cw2[0:R, ob * 128:(ob + 1) * 128],
                                     hsb[:, 0:1], start=True, stop=False)
                    nc.tensor.matmul(pc[:, 0:1],
                                     cw2[0:R, ob * 128:(ob + 1) * 128],
                                     hsb[:, 1:2], start=False, stop=True)
                    nc.scalar.activation(
                        out=ca[ob], in_=pc[:, 0:1],
                        func=mybir.ActivationFunctionType.Tanh, scale=0.5)
                    nc.vector.tensor_scalar(
                        ca[ob], ca[ob], 0.5, 0.5,
                        mybir.AluOpType.mult, mybir.AluOpType.add)
                # apply channel attention -> z_ca (bf16)
                zca = [zp.tile([128, N], BF16, tag=f"zca{ob}", name=f"zca{ob}")
                       for ob in range(2)]
                for ob in range(2):
                    if s == 1:
                        nc.scalar.activation(
                            out=zca[ob], in_=zt[ob],
                            func=mybir.ActivationFunctionType.Copy,
                            scale=ca[ob])
                    else:
                        nc.vector.tensor_scalar_mul(zca[ob], zt[ob], ca[ob])
                return zca

            def c_spat(s, zca):
                # spatial sum (avg path) via ones matmul (bf16)
                avg_row = sp.tile([1, N], BF16, tag="avg_row", name="avg_row",
                                  bufs=1)
                for nch in range(2):
                    psr = ps_a.tile([128, 512], F32, tag="pa", name="pa")
                    for kb in range(2):
                        nc.tensor.matmul(
                            psr[0:1, :],
                            ones_fr, zca[kb][:, nch * 512:(nch + 1) * 512],
                            start=(kb == 0), stop=(kb == 1))
                    nc.scalar.copy(
                        avg_row[:, nch * 512:(nch + 1) * 512], psr[0:1, :])
                # spatial max via TT max (bf16 2x) + gpsimd reduce
                m1 = zp.tile([128, N], BF16, tag="m1", name="m1", bufs=1)
                nc.vector.tensor_tensor(out=m1, in0=zca[0], in1=zca[1],
                                        op=mybir.AluOpType.max)
                rep = zp.tile([128, N], BF16, tag="rep", name="rep", bufs=1)
                nc.gpsimd.partition_all_reduce(rep, m1, channels=128,
                                               reduce_op=bass_isa.ReduceOp.max)
                # reshape rows [1, 1024] -> [32(y), 32(x)] via SBUF->SBUF DMA
                avgT = sp.tile([32, 38], BF16, tag="avgT", name="avgT")
                nc.gpsimd.memset(avgT, 0.0)
                nc.sync.dma_start(out=avgT[:, 3:35], in_=avg_row)
                maxT = sp.tile([32, 38], BF16, tag="maxT", name="maxT")
                nc.gpsimd.memset(maxT, 0.0)
                nc.sync.dma_start(out=maxT[:, 3:35], in_=rep[0:1, :])
                # 7x7 conv as 14 banded matmuls over y, x-shifts on free dim
                psa = ps_a.tile([128, 512], F32, tag="pa", name="pa")
                first = True
                for c2, inp in ((0, avgT), (1, maxT)):
                    for kx in range(7):
                        nc.tensor.matmul(
                            psa[0:32, 0:32],
                            wbd[:, c2 * 7 + kx, :],
                            inp[:, kx:kx + 32],
                            start=first, stop=(c2 == 1 and kx == 6))
                        first = False
                sasb = sp.tile([32, 32], BF16, tag="sasb", name="sasb")
                nc.scalar.activation(
                    out=sasb, in_=psa[0:32, 0:32],
                    func=mybir.ActivationFunctionType.Tanh, scale=0.5)
                nc.vector.tensor_scalar(
                    sasb, sasb, 0.5, 0.5,
                    mybir.AluOpType.mult, mybir.AluOpType.add)
                sarow = sp.tile([1, N], BF16, tag="sarow", name="sarow")
                nc.sync.dma_start(out=sarow, in_=sasb)
                sarep = zp.tile([128, N], BF16, tag="sarep", name="sarep",
                                bufs=1)
                nc.gpsimd.partition_broadcast(sarep, sarow, channels=128)
                return sarep

            def c_fin(s, zca, sarep):
                # final: out = relu(z_ca * sa + x); cb0 TTs on DVE, cb1 TTs
                # on Pool so both channel blocks run in parallel in the tail
                us = [op_.tile([128, N], BF16, tag=f"fin{cb}", name=f"fin{cb}")
                      for cb in range(2)]
                os_ = [op_.tile([128, N], F32, tag=f"fino{cb}",
                                name=f"fino{cb}") for cb in range(2)]
                for ch in range(2):
                    lo, hi = ch * 512, ch * 512 + 512
                    for cb in range(2):
                        eng = nc.gpsimd if (cb == 1 and ch == 1) else nc.vector
                        u, o = us[cb], os_[cb]
                        eng.tensor_tensor(
                            out=u[:, lo:hi], in0=zca[cb][:, lo:hi],
                            in1=sarep[:, lo:hi], op=mybir.AluOpType.mult)
                        eng.tensor_tensor(
                            out=u[:, lo:hi], in0=u[:, lo:hi],
                            in1=xt[s][cb][:, lo:hi], op=mybir.AluOpType.add)
                        nc.vector.tensor_scalar(
                            o[:, lo:hi], u[:, lo:hi], 0.0, None,
                            mybir.AluOpType.max)
                        nc.sync.dma_start(out=out[s, cb][:, lo:hi],
                                          in_=o[:, lo:hi])

            def phase_c(s):
                zt, cols = c_conv(s)
                zca = c_chan(s, zt, cols)
                sarep = c_spat(s, zca)
                c_fin(s, zca, sarep)

            phase_a(0)
            phase_b(0, extras=(lambda: a_conv(1), lambda: a_q(1),
                               lambda: a_k(1), lambda: a_vt(1)))
            c0_state = {}

            def x0():
                c0_state["zt"], c0_state["cols"] = c_conv(0)

            def x1():
                c0_state["zca"] = c_chan(0, c0_state["zt"], c0_state["cols"])

            def x2():
                c0_state["sarep"] = c_spat(0, c0_state["zca"])

            def x3():
                c_fin(0, c0_state["zca"], c0_state["sarep"])

            phase_b(1, extras=(x0, x1, x2, x3))
            phase_c(1)

    nc.compile()
    return nc


_NC_CACHE = None


def get_module():
    global _NC_CACHE
    if _NC_CACHE is None:
        _NC_CACHE = build_module()
    return _NC_CACHE


def prep_inputs(x, w1, bn1_g, bn1_b, bn1_m, bn1_v, wq, bq, wk, bk, wv, bv,
                gamma, w2, bn2_g, bn2_b, bn2_m, bn2_v, ca_w1, ca_w2, sa_w):
    """Host-side preprocessing -> per-core in_maps."""
    f64 = np.float64
    bf16 = ml_dtypes.bfloat16
    s1 = (bn1_g.astype(f64) / np.sqrt(bn1_v.astype(f64) + EPS))
    w1f = (s1[:, None] * w1.astype(f64)).astype(np.float32)
    sh1 = (bn1_b.astype(f64) - bn1_m.astype(f64) * s1).astype(np.float32)
    s2 = (bn2_g.astype(f64) / np.sqrt(bn2_v.astype(f64) + EPS))
    w2f = (s2[:, None] * w2.astype(f64)).astype(np.float32)
    g = float(gamma[0])
    wvg = (wv.astype(f64) * g).astype(np.float32)
    bvg = (bv.astype(f64) * g)
    # v-bias folds into the conv2 shift: softmax rows sum to 1, so the
    # attention output gets exactly +bvg per channel before conv2.
    sh2 = (bn2_b.astype(f64) - bn2_m.astype(f64) * s2
           + (s2[:, None] * w2.astype(f64)) @ bvg).astype(np.float32)

    def lhsT(w):  # [O, C] -> [2, 128, O] kb-blocked transpose, bf16
        return np.ascontiguousarray(w.T.reshape(2, 128, C)).astype(bf16)

    def lhsT_qk(w):
        wt = (w.astype(f64) * ALPHA).astype(np.float32).T  # [C_in, O]
        return np.ascontiguousarray(wt.reshape(2, 128, C)).astype(bf16)

    def bias_qk(b):
        bb = (b.astype(f64) * ALPHA).astype(np.float32)
        return np.ascontiguousarray(bb.reshape(2, 128, 1))

    base = {
        "w1fT": lhsT(w1f),
        "wqT": lhsT_qk(wq),
        "wkT": lhsT_qk(wk),
        "wvT": np.ascontiguousarray(wvg.T.reshape(2, 128, C)).astype(bf16),
        "w2fT": lhsT(w2f),
        "shift1": np.ascontiguousarray(sh1.reshape(2, 128, 1)),
        "bq_c": bias_qk(bq),
        "shift2": np.ascontiguousarray(sh2.reshape(2, 128, 1)),
    }
    # channel attention weights: caw1T [2, 128, 64]
    c1T = ca_w1.T.astype(np.float32)             # [C, R]
    caw1T = np.concatenate([c1T / float(N), c1T], axis=1)  # [C, 2R]
    base["caw1T"] = np.ascontiguousarray(caw1T.reshape(2, 128, 2 * R))
    c2T = ca_w2.T.astype(np.float32)             # [R, C]
    base["caw2T"] = np.ascontiguousarray(np.concatenate([c2T, c2T], axis=0))
    # spatial conv bands: wband[yi, c2*7+kx, yo] = w[c2, yi-yo+3, kx]
    wb = np.zeros((32, 14, 32), np.float32)
    for c2 in range(2):
        for kx in range(7):
            for yo in range(32):
                for ky in range(7):
                    yi = yo + ky - 3
                    if 0 <= yi < 32:
                        v = sa_w[0, c2, ky, kx]
                        if c2 == 0:
                            v = v / float(C)
                        wb[yi, c2 * 7 + kx, yo] = v
    base["wband"] = wb.astype(bf16)
    base["ones_in"] = np.ones((128, 1), bf16)

    xrf = x.reshape(B, C, N).astype(bf16)
    in_maps = []
    for core in range(NCORES):
        m = dict(base)
        m["xr"] = np.ascontiguousarray(
            xrf[core * SPC:(core + 1) * SPC].reshape(SPC, 2, 128, N))
        in_maps.append(m)
    return in_maps


def kernel(**inputs):
    nc = get_module()
    in_maps = prep_inputs(**inputs)
    res = run_bass_kernel_spmd(nc, in_maps, core_ids=list(range(NCORES)))
    outs = []
    for core in range(NCORES):
        o = res.results[core]["out"]  # [SPC, 2, 128, N]
        outs.append(o.reshape(SPC, C, H, W))
    return np.concatenate(outs, axis=0)


if __name__ == "__main__":
    nc = get_module()
    print("compiled ok")
